# revision 20
# baseline (speedup 1.0000x reference)
"""Distributed Trainium2 Bass kernel for the DenoisingNetwork GNN.

Strategy (8-way node/edge sharding, graph-parallel):
  * Nodes are assigned to 8 cores x NW windows of up to 124 nodes each,
    balanced so every (core, window) slot has nearly equal edge counts
    (LPT bin packing on destination degree).  Edges live with their dst.
  * Key algebra: since attention is a per-edge scalar,
        segment_sum(att * (relu(pre) @ Wm2)) = segment_sum(att*relu(pre)) @ Wm2
    so the only E-scale work is elementwise + a one-hot segment matmul.
  * Per-edge pre-activations decompose into node projections:
        pre = [h@W_i](dst) + [h@W_j](src) + [eemb@W_e + b](attr)
    The src part is gathered from an AllGather'ed per-layer table via
    dma_gather; the dst+attr parts are broadcast per-edge with a one-hot
    matmul on the TensorEngine (PE emits D[dst]+C[attr] straight to PSUM).
  * GRU + projections are node-level feature-major matmuls; the tail
    (readout heads) is replicated on every core after tiny AllReduces.

kernel(**inputs) takes the full-size numpy inputs, shards on host,
runs the SPMD Bass kernel on 8 NeuronCores, and returns
(node_probs [NT], edge_probs [M, ET]) like the reference.
"""

import heapq
import os
import sys

sys.path.insert(0, "/opt/trn_rl_repo")

import numpy as np
import ml_dtypes

import concourse.bass as bass
import concourse.bacc as bacc
import concourse.mybir as mybir
import concourse.tile as tile
from concourse import library_config
from concourse.vector_clock import ScopedClock

BF16 = mybir.dt.bfloat16
F32 = mybir.dt.float32
I16 = mybir.dt.int16
AF = mybir.ActivationFunctionType
OP = mybir.AluOpType
AX = mybir.AxisListType

nbf = ml_dtypes.bfloat16


# --------------------------------------------------------------------------
# Walrus in this container rejects CTRL instructions with >2 sync waits;
# Tile's tail drain aggregates one wait per active proc.  Chunk them.
class TileContextChunkedDrain(tile.TileContext):
    def _drain_and_barrier(self, tick_clock, wait_clock):
        probe = self.nc.sync.drain()
        wait_clock.add_sem_waits(
            probe.ins, ScopedClock({None: tick_clock.global_clock})
        )
        si = probe.ins.sync_info
        waits = list(si.on_wait or []) if si is not None else []
        if len(waits) > 1:
            si.on_wait = waits[:1]
            for w in waits[1:]:
                d = self.nc.sync.drain()
                dsi = d.ins.sync_info
                if dsi is None:
                    d.ins.sync_info = type(si)(on_wait=[w], on_update=[])
                else:
                    dsi.on_wait = [w]
        self.nc.all_engine_barrier()
        assert self.sems is not None
        popped = self.nc._tile_sem_poison_stack.pop()
        assert popped is self._sem_poison
        self.nc.clear_and_free_semaphores(list(self.sems.allocated().values()))
        self.nc.all_engine_barrier()


# --------------------------------------------------------------------------
class Cfg:
    def __init__(self, N, E, C, L, NT, ET, K, M, TPW, NW):
        self.N = N            # total nodes
        self.E = E            # total edges
        self.C = C            # cores
        self.L = L            # message passing layers
        self.NT = NT          # node types (+mask)
        self.ET = ET          # edge types (+mask)
        self.K = K            # mixture components
        self.M = M            # previous nodes
        self.H = 256
        self.WN = 124         # nodes per window
        self.NW = NW          # windows per core
        self.NL = NW * self.WN  # node slots per core
        self.R = C * self.NL    # global table rows
        self.TPW = TPW        # 128-edge tiles per window (even)
        self.EW = TPW * 128   # edge slots per window
        assert TPW % 2 == 0
        self.HT = TPW // 2    # tiles per half-window


def _lpt_assign(deg, C, NW, WN):
    """Assign nodes to C*NW slots (cap WN nodes each), balancing edge counts."""
    nslots = C * NW
    order = np.argsort(-deg, kind="stable")
    heap = [(0, 0, s) for s in range(nslots)]  # (edges, nodes, slot)
    heapq.heapify(heap)
    slot_of = np.empty(len(deg), np.int32)
    pos_of = np.empty(len(deg), np.int32)
    nodes_in = np.zeros(nslots, np.int64)
    spill = []
    for g in order:
        while True:
            e, n, s = heapq.heappop(heap)
            if n < WN:
                break
            spill.append((e, n, s))
        slot_of[g] = s
        pos_of[g] = n
        nodes_in[s] = n + 1
        heapq.heappush(heap, (e + int(deg[g]), n + 1, s))
        for item in spill:
            heapq.heappush(heap, item)
        spill.clear()
    return slot_of, pos_of


def preprocess(inputs, C=8, L=None):
    x_nodes = np.asarray(inputs["x_nodes"])
    edge_index = np.asarray(inputs["edge_index"])
    edge_attr = np.asarray(inputs["edge_attr"])
    target = int(np.asarray(inputs["target_node_idx"]))
    prev = np.asarray(inputs["previous_nodes"])
    node_emb = np.asarray(inputs["node_emb"], np.float32)
    eemb_t = np.asarray(inputs["edge_emb_table"], np.float32)
    mp = inputs["mp_params"]
    node_pred = inputs["node_pred"]
    edge_pred = inputs["edge_pred"]
    mix_pred = inputs["mix_pred"]

    N = x_nodes.shape[0]
    E = edge_index.shape[1]
    if L is None:
        L = len(mp)
    NT = node_emb.shape[0]
    ET = eemb_t.shape[0]
    K = np.asarray(edge_pred["W1"]).shape[0]
    M = prev.shape[0]
    H = 256
    WN = 124
    NW = -(-N // (C * WN))

    src, dst = edge_index[0], edge_index[1]
    deg = np.bincount(dst, minlength=N)
    slot_of, pos_of = _lpt_assign(deg, C, NW, WN)
    edges_per_slot = np.bincount(slot_of[dst], minlength=C * NW)
    TPW = max(2, int(-(-edges_per_slot.max() // 128)))
    if TPW % 2:
        TPW += 1
    cfg = Cfg(N, E, C, L, NT, ET, K, M, TPW, NW)
    NL, R, EW = cfg.NL, cfg.R, cfg.EW

    core_of = slot_of // NW
    win_of = slot_of % NW
    newrow = core_of.astype(np.int64) * NL + win_of * WN + pos_of

    # ---- per-edge placement
    e_core = core_of[dst]
    e_win = win_of[dst]
    e_pos = pos_of[dst]
    e_src_row = newrow[src]
    # sort edges by (core, window, pos) and lay out into padded slots
    eorder = np.lexsort((e_pos, e_win, e_core))
    sC, sW, sP = e_core[eorder], e_win[eorder], e_pos[eorder]
    sSrc, sAttr = e_src_row[eorder], edge_attr[eorder]
    # slot index within (core, win): cumulative count
    swkey = sC.astype(np.int64) * NW + sW
    slot_start = np.zeros(C * NW + 1, np.int64)
    np.cumsum(np.bincount(swkey, minlength=C * NW), out=slot_start[1:])
    j_in_slot = np.arange(E) - slot_start[swkey]

    qat = np.zeros((C, 128, NW * TPW * 128), np.float32)
    qf = np.zeros((C, 128, NW * TPW * 124), np.float32)
    sidx = np.zeros((C, NW * EW), np.int64)
    col = sW * EW + j_in_slot
    tno = j_in_slot >> 7
    rno = j_in_slot & 127
    qat[sC, sP, col] = 1.0
    qat[sC, 124 + sAttr, col] = 1.0
    qf[sC, rno, (sW * TPW + tno) * 124 + sP] = 1.0
    sidx[sC, col] = sSrc

    # wrapped int16 index layout for dma_gather: [128, n//16] replicated x8
    def wrap_idx(flat):
        n = flat.shape[0]
        w = flat.reshape(n // 16, 16).T
        return np.tile(w, (8, 1)).astype(np.int16)

    # ---- initial node features, feature-major per core
    h0 = node_emb[x_nodes]  # [N, H]
    h0_all = np.zeros((R, H), np.float32)
    h0_all[newrow] = h0

    # ---- weights
    def b16(x):
        return np.asarray(x, np.float32).astype(nbf)

    WBLOB_COLS = 2048 + 768 + 2 * 2304
    wblob = np.zeros((L, 128, WBLOB_COLS), np.float32)
    c_in = np.zeros((L, 4, NW * 512), np.float32)
    ba2s = []
    for l in range(L):
        p = {k: np.asarray(v, np.float32) for k, v in mp[l].items()}
        wcat = np.concatenate(
            [p["Wm1"][0:H], p["Wa1"][0:H], p["Wm1"][H : 2 * H], p["Wa1"][H : 2 * H]],
            axis=1,
        )  # [256, 1024] cols [D_m|D_a|S_m|S_a]
        wblob[l, :, 0:1024] = wcat[0:128]
        wblob[l, :, 1024:2048] = wcat[128:256]
        w2a = np.concatenate([p["Wm2"], p["bm2"][None, :]], axis=0)  # [257, 256]
        wblob[l, :, 2048:2304] = w2a[0:128]
        wblob[l, :, 2304:2560] = w2a[128:256]
        wblob[l, 0, 2560:2816] = w2a[256]
        for off, (wname, bname) in ((2816, ("Wih", "bih")), (5120, ("Whh", "bhh"))):
            wa = np.concatenate([p[wname], p[bname][None, :]], axis=0)  # [257, 768]
            wblob[l, :, off : off + 768] = wa[0:128]
            wblob[l, :, off + 768 : off + 1536] = wa[128:256]
            wblob[l, 0, off + 1536 : off + 2304] = wa[256]
        cm = eemb_t @ p["Wm1"][2 * H : 3 * H] + p["bm1"][None, :]
        ca = eemb_t @ p["Wa1"][2 * H : 3 * H] + p["ba1"][None, :]
        cful = np.concatenate([cm, ca], axis=1)  # [4, 512]
        c_in[l] = np.tile(cful[:, None, :], (1, NW, 1)).reshape(4, NW * 512)
        ba2s.append(float(p["ba2"].reshape(-1)[0]))
    wa2_tile = np.tile(
        np.asarray(mp[0]["Wa2"], np.float32).reshape(1, H), (128, 1)
    )  # per-layer below
    wa2_all = np.stack(
        [np.tile(np.asarray(mp[l]["Wa2"], np.float32).reshape(1, H), (128, 1)) for l in range(L)]
    )  # [L, 128, 256]

    # ---- tail weights blob
    npd = {k: np.asarray(v, np.float32) for k, v in node_pred.items()}
    epd = {k: np.asarray(v, np.float32) for k, v in edge_pred.items()}
    mpd = {k: np.asarray(v, np.float32) for k, v in mix_pred.items()}
    cols = []
    offs = {}

    def add_block(name, arr128):
        offs[name] = sum(a.shape[1] for a in cols)
        cols.append(arr128.astype(np.float32))

    def chunk_lhsT(w):  # [Kdim, Mdim] -> [128, (Kdim//128)*Mdim]
        kd, md = w.shape
        assert kd % 128 == 0
        return np.concatenate([w[i * 128 : (i + 1) * 128] for i in range(kd // 128)], axis=1)

    add_block("nW1", chunk_lhsT(npd["W1"]))          # 4*256
    add_block("nb1", npd["b1"].reshape(2, 128).T)     # [128,2]
    add_block("nW2", chunk_lhsT(npd["W2"]))           # 2*10
    add_block("mW1", chunk_lhsT(mpd["W1"]))           # 6*256
    add_block("mb1", mpd["b1"].reshape(2, 128).T)
    add_block("mW2", chunk_lhsT(mpd["W2"]))           # 2*20
    add_block("eb1", np.concatenate([epd["b1"][k].reshape(2, 128).T for k in range(K)], axis=1))
    add_block("eW2", np.concatenate([chunk_lhsT(epd["W2"][k]) for k in range(K)], axis=1))
    add_block("eW1", np.concatenate([chunk_lhsT(epd["W1"][k]) for k in range(K)], axis=1))
    tailblob = np.concatenate(cols, axis=1)
    small_cols = offs["eW1"]
    rowblob = np.zeros((1, 16 + 32 + 4 * K), np.float32)
    rowblob[0, 0:NT] = npd["b2"]
    rowblob[0, 16 : 16 + K] = mpd["b2"]
    rowblob[0, 48 : 48 + 4 * K] = epd["b2"].reshape(-1)
    roffs = {"nb2": 0, "mb2": 16, "eb2": 48}

    # ---- prev/target gather rows (target at slot 128 -> tile 1 partition 0)
    pidx = np.zeros(256, np.int64)
    pidx[:M] = newrow[prev]
    pidx[128] = newrow[target]

    meta = dict(cfg=cfg, ba2s=ba2s, toffs=offs, roffs=roffs,
                tail_cols=tailblob.shape[1], row_cols=rowblob.shape[1],
                small_cols=small_cols, wblob_cols=WBLOB_COLS)

    in_maps = []
    for c in range(C):
        blk = h0_all[c * NL : (c + 1) * NL]  # [NL, 256]
        h0T = np.concatenate([blk.T[0:128], blk.T[128:256]], axis=1)  # [128, 2*NL]
        m = {
            "h0T": np.ascontiguousarray(h0T, np.float32),
            "qat": np.ascontiguousarray(qat[c]).astype(nbf),
            "qf": np.ascontiguousarray(qf[c]).astype(nbf),
            "sidx": np.ascontiguousarray(
                np.concatenate([wrap_idx(sidx[c, w * EW : (w + 1) * EW]) for w in range(NW)], axis=1)
            ),
            "pidx": wrap_idx(pidx),
            "wblob": wblob.astype(nbf),
            "c_in": c_in.astype(nbf),
            "wa2t": wa2_all.astype(nbf),
            "tailblob": tailblob.astype(nbf),
            "rowblob": rowblob.astype(nbf),
        }
        in_maps.append(m)
    return in_maps, meta


# --------------------------------------------------------------------------
def build(meta):
    cfg = meta["cfg"]
    C, L, NW, NL, WN, TPW, HT, EW = (
        cfg.C, cfg.L, cfg.NW, cfg.NL, cfg.WN, cfg.TPW, cfg.HT, cfg.EW,
    )
    R, H, K, M, NT, ET = cfg.R, cfg.H, cfg.K, cfg.M, cfg.NT, cfg.ET
    HT = cfg.HT
    toffs, roffs = meta["toffs"], meta["roffs"]
    grp = [list(range(C))]

    nc = bacc.Bacc("TRN2", target_bir_lowering=False, debug=False, num_devices=C)
    # ---------- I/O
    h0T_d = nc.dram_tensor("h0T", [128, 2 * NL], F32, kind="ExternalInput")
    qat_d = nc.dram_tensor("qat", [128, NW * TPW * 128], BF16, kind="ExternalInput")
    qf_d = nc.dram_tensor("qf", [128, NW * TPW * 124], BF16, kind="ExternalInput")
    sidx_d = nc.dram_tensor("sidx", [128, NW * TPW * 8], I16, kind="ExternalInput")
    pidx_d = nc.dram_tensor("pidx", [128, 16], I16, kind="ExternalInput")
    wblob_d = nc.dram_tensor("wblob", [L, 128, meta["wblob_cols"]], BF16, kind="ExternalInput")
    cin_d = nc.dram_tensor("c_in", [L, 4, NW * 512], BF16, kind="ExternalInput")
    wa2_d = nc.dram_tensor("wa2t", [L, 128, 256], BF16, kind="ExternalInput")
    tail_d = nc.dram_tensor("tailblob", [128, meta["tail_cols"]], BF16, kind="ExternalInput")
    row_d = nc.dram_tensor("rowblob", [1, meta["row_cols"]], BF16, kind="ExternalInput")
    nprob_d = nc.dram_tensor("node_probs", [1, NT], F32, kind="ExternalOutput")
    dbg_h = nc.dram_tensor("dbg_h", [128, 2 * NL], F32, kind="ExternalOutput") if os.environ.get("BASSGNN_DBG") else None
    dbg_ei = nc.dram_tensor("dbg_ei", [128, 6 * (M + 1)], F32, kind="ExternalOutput") if os.environ.get("BASSGNN_DBG") else None
    dbg_mix = nc.dram_tensor("dbg_mix", [M + 1, K], F32, kind="ExternalOutput") if os.environ.get("BASSGNN_DBG") else None
    eprob_d = nc.dram_tensor("edge_probs", [M, ET], F32, kind="ExternalOutput")

    # internal DRAM (Shared addr space only supported for >4-core groups)
    shared = "Shared" if C > 4 else "Local"
    ag_in = nc.dram_tensor("ag_in", [NL, 512], BF16)
    s_full = nc.dram_tensor("s_full", [R, 512], BF16, addr_space=shared)
    hnm_in = nc.dram_tensor("hnm_in", [NL, 256], BF16)
    hnm_full = nc.dram_tensor("hnm_full", [R, 256], BF16, addr_space=shared)
    red_in = nc.dram_tensor("red_in", [128, 2], F32)
    red_out = nc.dram_tensor("red_out", [128, 2], F32, addr_space=shared)

    with TileContextChunkedDrain(nc) as tc:
        with (
            tc.tile_pool(name="per", bufs=1) as per,          # persistent
            tc.tile_pool(name="wgt", bufs=1) as wgt,          # per-layer weights
            tc.tile_pool(name="str", bufs=2) as stp,          # streamed Q / gathers
            tc.tile_pool(name="wrk", bufs=2) as wrk,          # edge work tensors
            tc.tile_pool(name="sml", bufs=2) as sml,          # small tiles
            tc.tile_pool(name="psA", bufs=2, space="PSUM") as psA,   # dc / proj
            tc.tile_pool(name="psT", bufs=2, space="PSUM") as psT,   # t window accum
            tc.tile_pool(name="psS", bufs=2, space="PSUM") as psS,   # small psums
        ):
            nidx_reg = nc.gpsimd.alloc_register("nidx")
            nc.gpsimd.reg_mov(nidx_reg, HT * 128)
            nidx_reg2 = nc.gpsimd.alloc_register("nidx2")
            nc.gpsimd.reg_mov(nidx_reg2, 256)

            hT = per.tile([128, 2, NL], F32, tag="hT")
            hTb = per.tile([128, 2, NL], BF16, tag="hTb")
            aggT = per.tile([128, 2, NL], BF16, tag="aggT")
            attsT = per.tile([1, NL], BF16, tag="attsT")
            D_sb = per.tile([128, NW, 512], BF16, tag="D_sb")
            ones = per.tile([1, NL], BF16, tag="ones")
            ident = per.tile([128, 128], BF16, tag="ident")
            sidx_sb = per.tile([128, NW * TPW * 8], I16, tag="sidx")
            wa2_sb = per.tile([128, 256], BF16, tag="wa2")

            nc.sync.dma_start(out=hT[:].rearrange("p c n -> p (c n)"), in_=h0T_d[:])
            nc.sync.dma_start(out=sidx_sb[:], in_=sidx_d[:])
            nc.vector.memset(ones[:], 1.0)
            nc.gpsimd.memset(ident[:], 0.0)
            from concourse.masks import make_identity
            make_identity(nc, ident[:], nomemset=True)

            for l in range(cfg.L):
                wb = wgt.tile([128, meta["wblob_cols"]], BF16, tag="wb")
                nc.sync.dma_start(out=wb[:], in_=wblob_d[l])
                nc.sync.dma_start(out=wa2_sb[:], in_=wa2_d[l])
                nc.sync.dma_start(out=D_sb[124:128, :, :].rearrange("p w d -> p (w d)"), in_=cin_d[l])
                # bf16 copy of h
                nc.vector.tensor_copy(out=hTb[:].rearrange("p c n -> p (c n)"),
                                      in_=hT[:].rearrange("p c n -> p (c n)"))

                # ---------- PROJ: P = h @ Wcat, node-major [124, 1024] per window
                for w in range(NW):
                    ns = slice(w * WN, (w + 1) * WN)
                    ps_d = psA.tile([124, 512], F32, tag="dc")
                    ps_s = psA.tile([124, 512], F32, tag="dc")
                    for kc in range(2):
                        lhs = hTb[:, kc, ns]
                        nc.tensor.matmul(ps_d[:], lhsT=lhs, rhs=wb[:, kc * 1024 : kc * 1024 + 512],
                                         start=(kc == 0), stop=(kc == 1))
                        nc.tensor.matmul(ps_s[:], lhsT=lhs, rhs=wb[:, kc * 1024 + 512 : (kc + 1) * 1024],
                                         start=(kc == 0), stop=(kc == 1))
                    nc.scalar.activation(out=D_sb[0:124, w, :], in_=ps_d[:], func=AF.Copy)
                    s_tmp = sml.tile([124, 512], BF16, tag="stmp")
                    nc.scalar.activation(out=s_tmp[:], in_=ps_s[:], func=AF.Copy)
                    nc.sync.dma_start(out=ag_in[w * WN : (w + 1) * WN, :], in_=s_tmp[:])

                # ---------- AllGather S table
                nc.gpsimd.collective_compute(
                    "AllGather", OP.bypass, replica_groups=grp,
                    ins=[ag_in[:, :].opt()], outs=[s_full[:, :].opt()],
                )

                # ---------- edge phase
                for w in range(NW):
                    ps_t = psT.tile([124, 257], F32, tag="t")
                    nc.vector.memset(ps_t[:], 0.0)
                    for hw in range(2):
                        t0 = hw * HT
                        sg = stp.tile([128, HT, 512], BF16, tag="sg")
                        nc.gpsimd.dma_gather(
                            out_ap=sg[:], in_ap=s_full[:, :],
                            idxs_ap=sidx_sb[:, (w * TPW + t0) * 8 : (w * TPW + t0 + HT) * 8],
                            num_idxs=HT * 128, num_idxs_reg=nidx_reg, elem_size=512,
                        )
                        qa = stp.tile([128, HT * 128], BF16, tag="qa")
                        nc.sync.dma_start(out=qa[:], in_=qat_d[:, (w * TPW + t0) * 128 : (w * TPW + t0 + HT) * 128])
                        qft = stp.tile([128, HT * 124], BF16, tag="qft")
                        nc.sync.dma_start(out=qft[:], in_=qf_d[:, (w * TPW + t0) * 124 : (w * TPW + t0 + HT) * 124])

                        pre = wrk.tile([128, HT, 512], BF16, tag="pre")
                        for t in range(HT):
                            ps_dc = psA.tile([128, 512], F32, tag="dc")
                            nc.tensor.matmul(ps_dc[:], lhsT=qa[:, t * 128 : (t + 1) * 128],
                                             rhs=D_sb[:, w, :], start=True, stop=True)
                            nc.scalar.activation(out=pre[:, t, :], in_=ps_dc[:], func=AF.Copy)
                        # pre += gathered src projections
                        nc.vector.scalar_tensor_tensor(
                            out=pre[:].rearrange("p t d -> p (t d)"),
                            in0=pre[:].rearrange("p t d -> p (t d)"), scalar=1.0,
                            in1=sg[:].rearrange("p t d -> p (t d)"),
                            op0=OP.mult, op1=OP.add)
                        # attention logits: sum over relu(pre_a) * wa2
                        tmpy = wrk.tile([128, HT, 256], BF16, tag="tmpy")
                        nc.vector.scalar_tensor_tensor(
                            out=tmpy[:], in0=pre[:, :, 256:512], scalar=0.0,
                            in1=wa2_sb[:].rearrange("p (t d) -> p t d", t=1).to_broadcast([128, HT, 256]),
                            op0=OP.max, op1=OP.mult)
                        logit = sml.tile([128, HT], F32, tag="logit")
                        nc.vector.tensor_reduce(out=logit[:], in_=tmpy[:], axis=AX.X, op=OP.add)
                        contrib = wrk.tile([128, HT, 257], BF16, tag="contrib")
                        nc.scalar.activation(out=contrib[:, :, 256:257], in_=logit[:].rearrange("p (t o) -> p t o", o=1),
                                             func=AF.Sigmoid, bias=meta["ba2s"][l], scale=1.0)
                        nc.vector.scalar_tensor_tensor(
                            out=contrib[:, :, 0:256], in0=pre[:, :, 0:256], scalar=0.0,
                            in1=contrib[:, :, 256:257].to_broadcast([128, HT, 256]),
                            op0=OP.max, op1=OP.mult)
                        for t in range(HT):
                            nc.tensor.matmul(ps_t[:], lhsT=qft[:, t * 124 : (t + 1) * 124],
                                             rhs=contrib[:, t, :], start=False, stop=(hw == 1 and t == HT - 1),
                                             skip_group_check=True)
                    # ---- window tail: t -> transpose -> agg
                    t_sb = sml.tile([124, 257], BF16, tag="tsb")
                    nc.scalar.activation(out=t_sb[:], in_=ps_t[:], func=AF.Copy)
                    taugT = sml.tile([128, 2, 124], BF16, tag="taugT")
                    for fc in range(2):
                        ps_tr = psS.tile([128, 124], BF16, tag="s")
                        nc.tensor.transpose(out=ps_tr[:], in_=t_sb[:, fc * 128 : (fc + 1) * 128],
                                            identity=ident[0:124, 0:124])
                        nc.scalar.activation(out=taugT[:, fc, :], in_=ps_tr[:], func=AF.Copy)
                    ps_ta = psS.tile([1, 124], BF16, tag="s")
                    nc.tensor.transpose(out=ps_ta[:], in_=t_sb[:, 256:257], identity=ident[0:124, 0:124])
                    nc.scalar.activation(out=attsT[:, w * WN : (w + 1) * WN], in_=ps_ta[:], func=AF.Copy)
                    # agg.T = Wm2aug.T @ [t.T ; attsum]
                    for mc in range(2):
                        ps_ag = psS.tile([128, 124], F32, tag="s")
                        ms = slice(mc * 128, (mc + 1) * 128)
                        nc.tensor.matmul(ps_ag[:], lhsT=wb[:, 2048:2304][:, ms], rhs=taugT[:, 0, :], start=True, stop=False)
                        nc.tensor.matmul(ps_ag[:], lhsT=wb[:, 2304:2560][:, ms], rhs=taugT[:, 1, :], start=False, stop=False)
                        nc.tensor.matmul(ps_ag[:], lhsT=wb[0:1, 2560:2816][:, ms],
                                         rhs=attsT[:, w * WN : (w + 1) * WN], start=False, stop=True)
                        nc.scalar.activation(out=aggT[:, mc, w * WN : (w + 1) * WN], in_=ps_ag[:], func=AF.Copy)

                # ---------- GRU (batched over node chunks)
                IH, HH = 2816, 5120
                NCH = -(-NL // 512)
                while NL % NCH:
                    NCH += 1
                CHW = NL // NCH
                assert CHW * NCH == NL and CHW <= 512

                def gemm_gate(ps, ms, rhs_a, rhs_h, a_only=False, h_only=False):
                    first = True
                    for woff, r2 in ((IH, rhs_a), (HH, rhs_h)):
                        if (a_only and woff == HH) or (h_only and woff == IH):
                            continue
                        nc.tensor.matmul(ps[:], lhsT=wb[:, woff : woff + 768][:, ms], rhs=r2[0],
                                         start=first, stop=False, skip_group_check=True)
                        nc.tensor.matmul(ps[:], lhsT=wb[:, woff + 768 : woff + 1536][:, ms], rhs=r2[1],
                                         start=False, stop=False, skip_group_check=True)
                        last = (woff == HH) or a_only or h_only
                        nc.tensor.matmul(ps[:], lhsT=wb[0:1, woff + 1536 : woff + 2304][:, ms], rhs=r2[2],
                                         start=False, stop=last, skip_group_check=True)
                        first = False

                for ch in range(NCH):
                    nsl = slice(ch * CHW, (ch + 1) * CHW)
                    rhs_a = (aggT[:, 0, nsl], aggT[:, 1, nsl], ones[:, nsl])
                    rhs_h = (hTb[:, 0, nsl], hTb[:, 1, nsl], ones[:, nsl])
                    rz_sb = sml.tile([128, 4, CHW], BF16, tag="rz")
                    for mc in range(4):  # r,z rows 0:512, both sides accumulated
                        ps = psA.tile([128, CHW], F32, tag="gru")
                        gemm_gate(ps, slice(mc * 128, (mc + 1) * 128), rhs_a, rhs_h)
                        nc.scalar.activation(out=rz_sb[:, mc, :], in_=ps[:], func=AF.Sigmoid)
                    for fc in range(2):  # n rows 512:768
                        ms = slice(512 + fc * 128, 512 + (fc + 1) * 128)
                        ps_in = psA.tile([128, CHW], F32, tag="gru")
                        gemm_gate(ps_in, ms, rhs_a, rhs_h, a_only=True)
                        ps_hn = psA.tile([128, CHW], F32, tag="gru")
                        gemm_gate(ps_hn, ms, rhs_a, rhs_h, h_only=True)
                        nn_sb = sml.tile([128, CHW], F32, tag="nn")
                        # nn = tanh(in + r*hn)
                        nc.vector.tensor_tensor(out=nn_sb[:], in0=rz_sb[:, fc, :], in1=ps_hn[:], op=OP.mult)
                        nc.vector.tensor_tensor(out=nn_sb[:], in0=nn_sb[:], in1=ps_in[:], op=OP.add)
                        nc.scalar.activation(out=nn_sb[:], in_=nn_sb[:], func=AF.Tanh)
                        # h' = nn + z*(h - nn)
                        hmn = sml.tile([128, CHW], F32, tag="hmn")
                        nc.vector.tensor_tensor(out=hmn[:], in0=hT[:, fc, nsl], in1=nn_sb[:], op=OP.subtract)
                        nc.vector.tensor_tensor(out=hmn[:], in0=hmn[:], in1=rz_sb[:, 2 + fc, :], op=OP.mult)
                        nc.vector.tensor_tensor(out=hT[:, fc, nsl], in0=hmn[:], in1=nn_sb[:], op=OP.add)

            if dbg_h is not None:
                nc.sync.dma_start(out=dbg_h[:, :], in_=hT[:].rearrange("p c n -> p (c n)"))
            # ================= tail =================
            tb = per.tile([128, meta["small_cols"]], BF16, tag="tail")
            nc.sync.dma_start(out=tb[:], in_=tail_d[:, 0 : meta["small_cols"]])
            rb = per.tile([1, meta["row_cols"]], BF16, tag="row")
            nc.sync.dma_start(out=rb[:], in_=row_d[:])
            pidx_sb = sml.tile([128, 16], I16, tag="pidx")
            nc.sync.dma_start(out=pidx_sb[:], in_=pidx_d[:])

            # node-major h (bf16) for prev-node gather, via PE transposes
            nc.vector.tensor_copy(out=hTb[:].rearrange("p c n -> p (c n)"),
                                  in_=hT[:].rearrange("p c n -> p (c n)"))
            hnm_sb = sml.tile([124, 2, 128], BF16, tag="hnm")
            for w in range(NW):
                for fc in range(2):
                    ps_tr = psS.tile([128, 128], BF16, tag="s")
                    nc.tensor.transpose(out=ps_tr[0:124, :], in_=hTb[:, fc, w * WN : (w + 1) * WN], identity=ident[:])
                    nc.scalar.activation(out=hnm_sb[:, fc, :], in_=ps_tr[0:124, :], func=AF.Copy)
                nc.sync.dma_start(out=hnm_in[w * WN : (w + 1) * WN, :],
                                  in_=hnm_sb[:].rearrange("p c d -> p (c d)"))
            # local sums for hG
            hsum = sml.tile([128, 2], F32, tag="hsum")
            nc.vector.tensor_reduce(out=hsum[:], in_=hT[:], axis=AX.X, op=OP.add)
            nc.sync.dma_start(out=red_in[:, :], in_=hsum[:])
            nc.gpsimd.collective_compute("AllGather", OP.bypass, replica_groups=grp,
                                         ins=[hnm_in[:, :].opt()], outs=[hnm_full[:, :].opt()])
            nc.gpsimd.collective_compute("AllReduce", OP.add, replica_groups=grp,
                                         ins=[red_in[:, :].opt()], outs=[red_out[:, :].opt()])
            hG = sml.tile([128, 2], F32, tag="hG")
            nc.sync.dma_start(out=hG[:], in_=red_out[:, :])
            gp = sml.tile([128, 2, 256], BF16, tag="gp")
            nc.gpsimd.dma_gather(out_ap=gp[:], in_ap=hnm_full[:, :], idxs_ap=pidx_sb[:],
                                 num_idxs=256, num_idxs_reg=nidx_reg2, elem_size=256)

            M2 = M + 1
            eiT = per.tile([128, 6, M2], BF16, tag="eiT")
            invN = 1.0 / cfg.N
            for fc in range(2):
                # hG rows (scaled by 1/N)
                nc.vector.tensor_scalar(out=eiT[:, fc, :], in0=hG[:, fc : fc + 1].to_broadcast([128, M2]),
                                        scalar1=invN, scalar2=None, op0=OP.mult)
                # t_emb rows: transpose gathered row M (target)
                ps_te = psS.tile([128, 1], BF16, tag="s")
                nc.tensor.transpose(out=ps_te[:], in_=gp[0:1, 1, fc * 128 : (fc + 1) * 128],
                                    identity=ident[0:1, 0:1])
                te = sml.tile([128, 1], BF16, tag="tecp")
                nc.scalar.activation(out=te[:], in_=ps_te[:], func=AF.Copy)
                nc.vector.tensor_scalar(out=eiT[:, 2 + fc, :], in0=te[:].to_broadcast([128, M2]),
                                        scalar1=1.0, scalar2=None, op0=OP.mult)
                # h_vj rows: transpose gathered rows 0:M
                ps_tv = psS.tile([128, M2], BF16, tag="s")
                nc.tensor.transpose(out=ps_tv[:, 0:M], in_=gp[0:M, 0, fc * 128 : (fc + 1) * 128],
                                    identity=ident[0:M, 0:M])
                nc.vector.memset(eiT[:, 4 + fc, :], 0.0)
                nc.scalar.activation(out=eiT[:, 4 + fc, 0:M], in_=ps_tv[:, 0:M], func=AF.Copy)

            def mlp_head(w1t, w1off, b1off, kchunks, outT):
                # outT [128, 2, M2] bf16 = relu(W1.T @ eiT + b1)
                for mc in range(2):
                    ps = psS.tile([128, M2], F32, tag="s")
                    for kc in range(kchunks):
                        nc.tensor.matmul(ps[:], lhsT=w1t[:, w1off + kc * 256 : w1off + kc * 256 + 256][:, mc * 128 : (mc + 1) * 128],
                                         rhs=eiT[:, kc, :], start=(kc == 0), stop=(kc == kchunks - 1))
                    nc.scalar.activation(out=outT[:, mc, :], in_=ps[:], func=AF.Relu,
                                         bias=tb[:, b1off + mc : b1off + mc + 1], scale=1.0)

            def softmax_free(x_sb, n_p, n_f):
                mx = sml.tile([n_p, 1], F32, tag="mx")
                nc.vector.tensor_reduce(out=mx[:], in_=x_sb[:], axis=AX.X, op=OP.max, negate=True)
                se = sml.tile([n_p, 1], F32, tag="se")
                nc.scalar.activation(out=x_sb[:], in_=x_sb[:], func=AF.Exp, bias=mx[:], scale=1.0, accum_out=se[:])
                nc.vector.reciprocal(out=se[:], in_=se[:])
                nc.vector.tensor_scalar(out=x_sb[:], in0=x_sb[:], scalar1=se[:], scalar2=None, op0=OP.mult)

            if dbg_ei is not None:
                nc.gpsimd.dma_start(out=dbg_ei[:, :], in_=eiT[:].rearrange("p c n -> p (c n)"))
            # ---- mix head
            mixT = sml.tile([128, 2, M2], BF16, tag="mixT")
            mlp_head(tb, toffs["mW1"], toffs["mb1"], 6, mixT)
            mixl = sml.tile([M2, K], F32, tag="mixl")
            ps_ml = psS.tile([M2, K], F32, tag="s")
            for mc in range(2):
                nc.tensor.matmul(ps_ml[:], lhsT=mixT[:, mc, :], rhs=tb[:, toffs["mW2"] + mc * K : toffs["mW2"] + (mc + 1) * K],
                                 start=(mc == 0), stop=False, skip_group_check=True)
            nc.tensor.matmul(ps_ml[:], lhsT=ones[:, 0:M2], rhs=rb[:, roffs["mb2"] : roffs["mb2"] + K],
                             start=False, stop=True, skip_group_check=True)
            nc.scalar.activation(out=mixl[:], in_=ps_ml[:], func=AF.Copy)
            softmax_free(mixl, M2, K)

            if dbg_mix is not None:
                nc.sync.dma_start(out=dbg_mix[:, :], in_=mixl[:])
            # ---- edge heads
            acc = sml.tile([M2, ET], F32, tag="acc")
            nc.vector.memset(acc[:], 0.0)
            hkT = sml.tile([128, 2, M2], BF16, tag="hkT")
            for k in range(K):
                ekw = stp.tile([128, 6 * 256], BF16, tag="ekw")
                nc.sync.dma_start(out=ekw[:], in_=tail_d[:, toffs["eW1"] + k * 6 * 256 : toffs["eW1"] + (k + 1) * 6 * 256])
                mlp_head(ekw, 0, toffs["eb1"] + k * 2, 6, hkT)
                ps_el = psS.tile([M2, ET], F32, tag="s")
                for mc in range(2):
                    nc.tensor.matmul(ps_el[:], lhsT=hkT[:, mc, :],
                                     rhs=tb[:, toffs["eW2"] + (k * 2 + mc) * ET : toffs["eW2"] + (k * 2 + mc + 1) * ET],
                                     start=(mc == 0), stop=False, skip_group_check=True)
                nc.tensor.matmul(ps_el[:], lhsT=ones[:, 0:M2],
                                 rhs=rb[:, roffs["eb2"] + k * ET : roffs["eb2"] + (k + 1) * ET],
                                 start=False, stop=True, skip_group_check=True)
                ekl = sml.tile([M2, ET], F32, tag="ekl")
                nc.scalar.activation(out=ekl[:], in_=ps_el[:], func=AF.Copy)
                softmax_free(ekl, M2, ET)
                nc.vector.scalar_tensor_tensor(out=acc[:], in0=ekl[:], scalar=mixl[:, k : k + 1],
                                               in1=acc[:], op0=OP.mult, op1=OP.add)
            nc.sync.dma_start(out=eprob_d[:, :], in_=acc[0:M, :])

            # ---- node head
            niT = sml.tile([128, 4], BF16, tag="niT")
            for fc in range(2):
                nc.vector.tensor_scalar(out=niT[:, fc : fc + 1], in0=hG[:, fc : fc + 1],
                                        scalar1=invN, scalar2=None, op0=OP.mult)
                nc.vector.tensor_copy(out=niT[:, 2 + fc : 3 + fc], in_=eiT[:, 2 + fc, 0:1])
            nhT = sml.tile([128, 2], BF16, tag="nhT")
            for mc in range(2):
                ps = psS.tile([128, 1], F32, tag="s")
                for kc in range(4):
                    nc.tensor.matmul(ps[:], lhsT=tb[:, toffs["nW1"] + kc * 256 : toffs["nW1"] + kc * 256 + 256][:, mc * 128 : (mc + 1) * 128],
                                     rhs=niT[:, kc : kc + 1], start=(kc == 0), stop=(kc == 3))
                nc.scalar.activation(out=nhT[:, mc : mc + 1], in_=ps[:], func=AF.Relu,
                                     bias=tb[:, toffs["nb1"] + mc : toffs["nb1"] + mc + 1], scale=1.0)
            ps_nl = psS.tile([1, NT], F32, tag="s")
            for mc in range(2):
                nc.tensor.matmul(ps_nl[:], lhsT=nhT[:, mc : mc + 1],
                                 rhs=tb[:, toffs["nW2"] + mc * NT : toffs["nW2"] + (mc + 1) * NT],
                                 start=(mc == 0), stop=False, skip_group_check=True)
            nc.tensor.matmul(ps_nl[:], lhsT=ones[:, 0:1], rhs=rb[:, roffs["nb2"] : roffs["nb2"] + NT],
                             start=False, stop=True, skip_group_check=True)
            nl_sb = sml.tile([1, NT], F32, tag="nlsb")
            nc.scalar.activation(out=nl_sb[:], in_=ps_nl[:], func=AF.Copy)
            softmax_free(nl_sb, 1, NT)
            nc.sync.dma_start(out=nprob_d[:, :], in_=nl_sb[:])

    nc.compile()
    return nc


# --------------------------------------------------------------------------
_LAST_RESULTS = {}


def _run_spmd_timed(nc, in_maps, n_cores, reps=3):
    """Mirror bass2jax.run_bass_via_pjrt's multi-core path, but jit once,
    device_put inputs once, and wall-clock steady-state executions."""
    import time
    import jax
    from jax.sharding import Mesh, PartitionSpec, NamedSharding
    from jax.experimental.shard_map import shard_map
    import concourse.mybir as mybir_
    from concourse import bass2jax

    bass2jax.install_neuronx_cc_hook()
    partition_name = nc.partition_id_tensor.name if nc.partition_id_tensor else None
    in_names, out_names, out_avals, zero_shapes = [], [], [], []
    for alloc in nc.m.functions[0].allocations:
        if not isinstance(alloc, mybir_.MemoryLocationSet):
            continue
        name = alloc.memorylocations[0].name
        if alloc.kind == "ExternalInput":
            if name != partition_name:
                in_names.append(name)
        elif alloc.kind == "ExternalOutput":
            out_names.append(name)
            shape = tuple(alloc.tensor_shape)
            dtype = mybir_.dt.np(alloc.dtype)
            out_avals.append(jax.core.ShapedArray(shape, dtype))
            zero_shapes.append((shape, dtype))
    n_params = len(in_names)
    n_outs = len(out_avals)
    all_names = list(in_names) + out_names + ([partition_name] if partition_name else [])
    donate = tuple(range(n_params, n_params + n_outs))

    def _body(*args):
        operands = list(args)
        if partition_name is not None:
            operands.append(bass2jax.partition_id_tensor())
        return tuple(bass2jax._bass_exec_p.bind(
            *operands, out_avals=tuple(out_avals), in_names=tuple(all_names),
            out_names=tuple(out_names), lowering_input_output_aliases=(),
            sim_require_finite=True, sim_require_nnan=True, nc=nc))

    devices = jax.devices()[:n_cores]
    mesh = Mesh(np.asarray(devices), ("core",))
    spec = PartitionSpec("core")
    sharded = jax.jit(
        shard_map(_body, mesh=mesh, in_specs=(spec,) * (n_params + n_outs),
                  out_specs=(spec,) * n_outs, check_rep=False),
        donate_argnums=donate, keep_unused=True)
    concat_in = [
        np.concatenate([np.asarray(in_maps[c][nm]) for c in range(n_cores)], axis=0)
        for nm in in_names
    ]
    sh = NamedSharding(mesh, spec)
    dev_in = [jax.device_put(a, sh) for a in concat_in]

    def zeros():
        return [jax.device_put(np.zeros((n_cores * s[0], *s[1:]), d), sh)
                for s, d in zero_shapes]

    outs = sharded(*dev_in, *zeros())
    jax.block_until_ready(outs)
    times = []
    for _ in range(reps):
        z = zeros()
        jax.block_until_ready(z)
        t0 = time.perf_counter()
        outs = sharded(*dev_in, *z)
        jax.block_until_ready(outs)
        times.append(time.perf_counter() - t0)
    results = [
        {nm: np.asarray(outs[i]).reshape(n_cores, *zero_shapes[i][0])[c]
         for i, nm in enumerate(out_names)}
        for c in range(n_cores)
    ]
    return results, min(times)


def kernel(**inputs):
    from concourse.bass_utils import run_bass_kernel_spmd

    C = 8
    in_maps, meta = preprocess(inputs, C=C)
    nc = build(meta)
    if os.environ.get("BASSGNN_TIME", "0") == "1":
        results, secs = _run_spmd_timed(nc, in_maps, C)
        _LAST_RESULTS["exec_time_ns"] = secs * 1e9
        node_probs = np.asarray(results[0]["node_probs"]).reshape(-1)
        edge_probs = np.asarray(results[0]["edge_probs"])
        return node_probs, edge_probs
    trace = os.environ.get("BASSGNN_TRACE", "0") == "1"
    res = run_bass_kernel_spmd(nc, in_maps, core_ids=list(range(C)), trace=trace)
    _LAST_RESULTS["res"] = res
    node_probs = np.asarray(res.results[0]["node_probs"]).reshape(-1)
    edge_probs = np.asarray(res.results[0]["edge_probs"])
    return node_probs, edge_probs


# revision 27
# speedup vs baseline: 1.1167x; 1.1167x over previous
"""Distributed Trainium2 Bass kernel for the DenoisingNetwork GNN.

Strategy (8-way node/edge sharding, graph-parallel):
  * Nodes are assigned to 8 cores x NW windows of up to 124 nodes each,
    balanced so every (core, window) slot has nearly equal edge counts
    (LPT bin packing on destination degree).  Edges live with their dst.
  * Key algebra: since attention is a per-edge scalar,
        segment_sum(att * (relu(pre) @ Wm2)) = segment_sum(att*relu(pre)) @ Wm2
    so the only E-scale work is elementwise + a one-hot segment matmul.
  * Per-edge pre-activations decompose into node projections:
        pre = [h@W_i](dst) + [h@W_j](src) + [eemb@W_e + b](attr)
    The src part is gathered from an AllGather'ed per-layer table via
    dma_gather; the dst+attr parts are broadcast per-edge with a one-hot
    matmul on the TensorEngine (PE emits D[dst]+C[attr] straight to PSUM).
  * GRU + projections are node-level feature-major matmuls; the tail
    (readout heads) is replicated on every core after tiny AllReduces.

kernel(**inputs) takes the full-size numpy inputs, shards on host,
runs the SPMD Bass kernel on 8 NeuronCores, and returns
(node_probs [NT], edge_probs [M, ET]) like the reference.
"""

import heapq
import os
import sys

sys.path.insert(0, "/opt/trn_rl_repo")

import numpy as np
import ml_dtypes

import concourse.bass as bass
import concourse.bacc as bacc
import concourse.mybir as mybir
import concourse.tile as tile
from concourse import library_config
from concourse.vector_clock import ScopedClock

BF16 = mybir.dt.bfloat16
F32 = mybir.dt.float32
I16 = mybir.dt.int16
AF = mybir.ActivationFunctionType
OP = mybir.AluOpType
AX = mybir.AxisListType

nbf = ml_dtypes.bfloat16


# --------------------------------------------------------------------------
# Walrus in this container rejects CTRL instructions with >2 sync waits;
# Tile's tail drain aggregates one wait per active proc.  Chunk them.
class TileContextChunkedDrain(tile.TileContext):
    def _drain_and_barrier(self, tick_clock, wait_clock):
        probe = self.nc.sync.drain()
        wait_clock.add_sem_waits(
            probe.ins, ScopedClock({None: tick_clock.global_clock})
        )
        si = probe.ins.sync_info
        waits = list(si.on_wait or []) if si is not None else []
        if len(waits) > 1:
            si.on_wait = waits[:1]
            for w in waits[1:]:
                d = self.nc.sync.drain()
                dsi = d.ins.sync_info
                if dsi is None:
                    d.ins.sync_info = type(si)(on_wait=[w], on_update=[])
                else:
                    dsi.on_wait = [w]
        self.nc.all_engine_barrier()
        assert self.sems is not None
        popped = self.nc._tile_sem_poison_stack.pop()
        assert popped is self._sem_poison
        self.nc.clear_and_free_semaphores(list(self.sems.allocated().values()))
        self.nc.all_engine_barrier()


# --------------------------------------------------------------------------
class Cfg:
    def __init__(self, N, E, C, L, NT, ET, K, M, TPW, NW):
        self.N = N            # total nodes
        self.E = E            # total edges
        self.C = C            # cores
        self.L = L            # message passing layers
        self.NT = NT          # node types (+mask)
        self.ET = ET          # edge types (+mask)
        self.K = K            # mixture components
        self.M = M            # previous nodes
        self.H = 256
        self.WN = 124         # nodes per window
        self.NW = NW          # windows per core
        self.NL = NW * self.WN  # node slots per core
        self.R = C * self.NL    # global table rows
        self.TPW = TPW        # 128-edge tiles per window (even)
        self.EW = TPW * 128   # edge slots per window
        assert TPW % 2 == 0
        self.HT = TPW // 2    # tiles per half-window


def _lpt_assign(deg, C, NW, WN):
    """Assign nodes to C*NW slots (cap WN nodes each), balancing edge counts."""
    nslots = C * NW
    order = np.argsort(-deg, kind="stable")
    heap = [(0, 0, s) for s in range(nslots)]  # (edges, nodes, slot)
    heapq.heapify(heap)
    slot_of = np.empty(len(deg), np.int32)
    pos_of = np.empty(len(deg), np.int32)
    nodes_in = np.zeros(nslots, np.int64)
    spill = []
    for g in order:
        while True:
            e, n, s = heapq.heappop(heap)
            if n < WN:
                break
            spill.append((e, n, s))
        slot_of[g] = s
        pos_of[g] = n
        nodes_in[s] = n + 1
        heapq.heappush(heap, (e + int(deg[g]), n + 1, s))
        for item in spill:
            heapq.heappush(heap, item)
        spill.clear()
    return slot_of, pos_of


def preprocess(inputs, C=8, L=None):
    x_nodes = np.asarray(inputs["x_nodes"])
    edge_index = np.asarray(inputs["edge_index"])
    edge_attr = np.asarray(inputs["edge_attr"])
    target = int(np.asarray(inputs["target_node_idx"]))
    prev = np.asarray(inputs["previous_nodes"])
    node_emb = np.asarray(inputs["node_emb"], np.float32)
    eemb_t = np.asarray(inputs["edge_emb_table"], np.float32)
    mp = inputs["mp_params"]
    node_pred = inputs["node_pred"]
    edge_pred = inputs["edge_pred"]
    mix_pred = inputs["mix_pred"]

    N = x_nodes.shape[0]
    E = edge_index.shape[1]
    if L is None:
        L = len(mp)
    NT = node_emb.shape[0]
    ET = eemb_t.shape[0]
    K = np.asarray(edge_pred["W1"]).shape[0]
    M = prev.shape[0]
    H = 256
    WN = 124
    NW = -(-N // (C * WN))

    src, dst = edge_index[0], edge_index[1]
    deg = np.bincount(dst, minlength=N)
    slot_of, pos_of = _lpt_assign(deg, C, NW, WN)
    edges_per_slot = np.bincount(slot_of[dst], minlength=C * NW)
    TPW = max(2, int(-(-edges_per_slot.max() // 128)))
    if TPW % 2:
        TPW += 1
    cfg = Cfg(N, E, C, L, NT, ET, K, M, TPW, NW)
    NL, R, EW = cfg.NL, cfg.R, cfg.EW

    core_of = slot_of // NW
    win_of = slot_of % NW
    newrow = core_of.astype(np.int64) * NL + win_of * WN + pos_of

    # ---- per-edge placement
    e_core = core_of[dst]
    e_win = win_of[dst]
    e_pos = pos_of[dst]
    e_src_row = newrow[src]
    # sort edges by (core, window, pos) and lay out into padded slots
    eorder = np.lexsort((e_pos, e_win, e_core))
    sC, sW, sP = e_core[eorder], e_win[eorder], e_pos[eorder]
    sSrc, sAttr = e_src_row[eorder], edge_attr[eorder]
    # slot index within (core, win): cumulative count
    swkey = sC.astype(np.int64) * NW + sW
    slot_start = np.zeros(C * NW + 1, np.int64)
    np.cumsum(np.bincount(swkey, minlength=C * NW), out=slot_start[1:])
    j_in_slot = np.arange(E) - slot_start[swkey]

    qat = np.zeros((C, 128, NW * TPW * 128), np.float32)
    qf = np.zeros((C, 128, NW * TPW * 124), np.float32)
    sidx = np.zeros((C, NW * EW), np.int64)
    col = sW * EW + j_in_slot
    tno = j_in_slot >> 7
    rno = j_in_slot & 127
    qat[sC, sP, col] = 1.0
    qat[sC, 124 + sAttr, col] = 1.0
    qf[sC, rno, (sW * TPW + tno) * 124 + sP] = 1.0
    sidx[sC, col] = sSrc

    # wrapped int16 index layout for dma_gather: [128, n//16] replicated x8
    def wrap_idx(flat):
        n = flat.shape[0]
        w = flat.reshape(n // 16, 16).T
        return np.tile(w, (8, 1)).astype(np.int16)

    # ---- initial node features, feature-major per core
    h0 = node_emb[x_nodes]  # [N, H]
    h0_all = np.zeros((R, H), np.float32)
    h0_all[newrow] = h0

    # ---- weights
    def b16(x):
        return np.asarray(x, np.float32).astype(nbf)

    WBLOB_COLS = 2048 + 768 + 2 * 2304
    wblob = np.zeros((L, 128, WBLOB_COLS), np.float32)
    c_in = np.zeros((L, 4, NW * 512), np.float32)
    ba2s = []
    for l in range(L):
        p = {k: np.asarray(v, np.float32) for k, v in mp[l].items()}
        wcat = np.concatenate(
            [p["Wm1"][0:H], p["Wa1"][0:H], p["Wm1"][H : 2 * H], p["Wa1"][H : 2 * H]],
            axis=1,
        )  # [256, 1024] cols [D_m|D_a|S_m|S_a]
        wblob[l, :, 0:1024] = wcat[0:128]
        wblob[l, :, 1024:2048] = wcat[128:256]
        w2a = np.concatenate([p["Wm2"], p["bm2"][None, :]], axis=0)  # [257, 256]
        wblob[l, :, 2048:2304] = w2a[0:128]
        wblob[l, :, 2304:2560] = w2a[128:256]
        wblob[l, 0, 2560:2816] = w2a[256]
        for off, (wname, bname) in ((2816, ("Wih", "bih")), (5120, ("Whh", "bhh"))):
            wa = np.concatenate([p[wname], p[bname][None, :]], axis=0)  # [257, 768]
            wblob[l, :, off : off + 768] = wa[0:128]
            wblob[l, :, off + 768 : off + 1536] = wa[128:256]
            wblob[l, 0, off + 1536 : off + 2304] = wa[256]
        cm = eemb_t @ p["Wm1"][2 * H : 3 * H] + p["bm1"][None, :]
        ca = eemb_t @ p["Wa1"][2 * H : 3 * H] + p["ba1"][None, :]
        cful = np.concatenate([cm, ca], axis=1)  # [4, 512]
        c_in[l] = np.tile(cful[:, None, :], (1, NW, 1)).reshape(4, NW * 512)
        ba2s.append(float(p["ba2"].reshape(-1)[0]))
    wa2_tile = np.tile(
        np.asarray(mp[0]["Wa2"], np.float32).reshape(1, H), (128, 1)
    )  # per-layer below
    wa2_all = np.stack(
        [np.tile(np.asarray(mp[l]["Wa2"], np.float32).reshape(1, H), (128, 1)) for l in range(L)]
    )  # [L, 128, 256]

    # ---- tail weights blob
    npd = {k: np.asarray(v, np.float32) for k, v in node_pred.items()}
    epd = {k: np.asarray(v, np.float32) for k, v in edge_pred.items()}
    mpd = {k: np.asarray(v, np.float32) for k, v in mix_pred.items()}
    cols = []
    offs = {}

    def add_block(name, arr128):
        offs[name] = sum(a.shape[1] for a in cols)
        cols.append(arr128.astype(np.float32))

    def chunk_lhsT(w):  # [Kdim, Mdim] -> [128, (Kdim//128)*Mdim]
        kd, md = w.shape
        assert kd % 128 == 0
        return np.concatenate([w[i * 128 : (i + 1) * 128] for i in range(kd // 128)], axis=1)

    add_block("nW1", chunk_lhsT(npd["W1"]))          # 4*256
    add_block("nb1", npd["b1"].reshape(2, 128).T)     # [128,2]
    add_block("nW2", chunk_lhsT(npd["W2"]))           # 2*10
    add_block("mW1", chunk_lhsT(mpd["W1"]))           # 6*256
    add_block("mb1", mpd["b1"].reshape(2, 128).T)
    add_block("mW2", chunk_lhsT(mpd["W2"]))           # 2*20
    add_block("eb1", np.concatenate([epd["b1"][k].reshape(2, 128).T for k in range(K)], axis=1))
    add_block("eW2", np.concatenate([chunk_lhsT(epd["W2"][k]) for k in range(K)], axis=1))
    add_block("eW1", np.concatenate([chunk_lhsT(epd["W1"][k]) for k in range(K)], axis=1))
    tailblob = np.concatenate(cols, axis=1)
    small_cols = offs["eW1"]
    rowblob = np.zeros((1, 16 + 32 + 4 * K), np.float32)
    rowblob[0, 0:NT] = npd["b2"]
    rowblob[0, 16 : 16 + K] = mpd["b2"]
    rowblob[0, 48 : 48 + 4 * K] = epd["b2"].reshape(-1)
    roffs = {"nb2": 0, "mb2": 16, "eb2": 48}

    # ---- prev/target gather rows (target at slot 128 -> tile 1 partition 0)
    pidx = np.zeros(256, np.int64)
    pidx[:M] = newrow[prev]
    pidx[128] = newrow[target]

    meta = dict(cfg=cfg, ba2s=ba2s, toffs=offs, roffs=roffs,
                tail_cols=tailblob.shape[1], row_cols=rowblob.shape[1],
                small_cols=small_cols, wblob_cols=WBLOB_COLS)

    in_maps = []
    for c in range(C):
        blk = h0_all[c * NL : (c + 1) * NL]  # [NL, 256]
        h0T = np.concatenate([blk.T[0:128], blk.T[128:256]], axis=1)  # [128, 2*NL]
        m = {
            "h0T": np.ascontiguousarray(h0T, np.float32),
            "qat": np.ascontiguousarray(qat[c]).astype(nbf),
            "qf": np.ascontiguousarray(qf[c]).astype(nbf),
            "sidx": np.ascontiguousarray(
                np.concatenate([wrap_idx(sidx[c, w * EW : (w + 1) * EW]) for w in range(NW)], axis=1)
            ),
            "pidx": wrap_idx(pidx),
            "wblob": wblob.astype(nbf),
            "c_in": c_in.astype(nbf),
            "wa2t": wa2_all.astype(nbf),
            "tailblob": tailblob.astype(nbf),
            "rowblob": rowblob.astype(nbf),
        }
        in_maps.append(m)
    return in_maps, meta


# --------------------------------------------------------------------------
def build(meta):
    cfg = meta["cfg"]
    C, L, NW, NL, WN, TPW, HT, EW = (
        cfg.C, cfg.L, cfg.NW, cfg.NL, cfg.WN, cfg.TPW, cfg.HT, cfg.EW,
    )
    R, H, K, M, NT, ET = cfg.R, cfg.H, cfg.K, cfg.M, cfg.NT, cfg.ET
    HT = cfg.HT
    ABL = os.environ.get("BASSGNN_ABLATE", "")
    toffs, roffs = meta["toffs"], meta["roffs"]
    grp = [list(range(C))]

    nc = bacc.Bacc("TRN2", target_bir_lowering=False, debug=False, num_devices=C,
                   dynamic_dma_scratch_size=65536)
    # ---------- I/O
    h0T_d = nc.dram_tensor("h0T", [128, 2 * NL], F32, kind="ExternalInput")
    qat_d = nc.dram_tensor("qat", [128, NW * TPW * 128], BF16, kind="ExternalInput")
    qf_d = nc.dram_tensor("qf", [128, NW * TPW * 124], BF16, kind="ExternalInput")
    sidx_d = nc.dram_tensor("sidx", [128, NW * TPW * 8], I16, kind="ExternalInput")
    pidx_d = nc.dram_tensor("pidx", [128, 16], I16, kind="ExternalInput")
    wblob_d = nc.dram_tensor("wblob", [L, 128, meta["wblob_cols"]], BF16, kind="ExternalInput")
    cin_d = nc.dram_tensor("c_in", [L, 4, NW * 512], BF16, kind="ExternalInput")
    wa2_d = nc.dram_tensor("wa2t", [L, 128, 256], BF16, kind="ExternalInput")
    tail_d = nc.dram_tensor("tailblob", [128, meta["tail_cols"]], BF16, kind="ExternalInput")
    row_d = nc.dram_tensor("rowblob", [1, meta["row_cols"]], BF16, kind="ExternalInput")
    nprob_d = nc.dram_tensor("node_probs", [1, NT], F32, kind="ExternalOutput")
    dbg_h = nc.dram_tensor("dbg_h", [128, 2 * NL], F32, kind="ExternalOutput") if os.environ.get("BASSGNN_DBG") else None
    dbg_ei = nc.dram_tensor("dbg_ei", [128, 6 * (M + 1)], F32, kind="ExternalOutput") if os.environ.get("BASSGNN_DBG") else None
    dbg_mix = nc.dram_tensor("dbg_mix", [M + 1, K], F32, kind="ExternalOutput") if os.environ.get("BASSGNN_DBG") else None
    eprob_d = nc.dram_tensor("edge_probs", [M, ET], F32, kind="ExternalOutput")

    # internal DRAM (Shared addr space only supported for >4-core groups)
    shared = "Shared" if C > 4 else "Local"
    ag_in = nc.dram_tensor("ag_in", [NL, 512], BF16)
    s_full = nc.dram_tensor("s_full", [R, 512], BF16, addr_space=shared)
    hnm_in = nc.dram_tensor("hnm_in", [NL, 256], BF16)
    hnm_full = nc.dram_tensor("hnm_full", [R, 256], BF16, addr_space=shared)
    red_in = nc.dram_tensor("red_in", [128, 2], F32)
    red_out = nc.dram_tensor("red_out", [128, 2], F32, addr_space=shared)

    with TileContextChunkedDrain(nc) as tc:
        with (
            tc.tile_pool(name="per", bufs=1) as per,          # persistent
            tc.tile_pool(name="wgt", bufs=1) as wgt,          # per-layer weights
            tc.tile_pool(name="str", bufs=2) as stp,          # streamed Q / gathers
            tc.tile_pool(name="wrk", bufs=2) as wrk,          # edge work tensors
            tc.tile_pool(name="sml", bufs=2) as sml,          # small tiles
            tc.tile_pool(name="psA", bufs=2, space="PSUM") as psA,   # dc / proj
            tc.tile_pool(name="psT", bufs=2, space="PSUM") as psT,   # t window accum
            tc.tile_pool(name="psS", bufs=2, space="PSUM") as psS,   # small psums
        ):
            nidx_reg = nc.gpsimd.alloc_register("nidx")
            nc.gpsimd.reg_mov(nidx_reg, HT * 128)
            nidx_reg2 = nc.gpsimd.alloc_register("nidx2")
            nc.gpsimd.reg_mov(nidx_reg2, 256)

            hT = per.tile([128, 2, NL], F32, tag="hT")
            hTb = per.tile([128, 2, NL], BF16, tag="hTb")
            aggT = per.tile([128, 2, NL], BF16, tag="aggT")
            D_sb = per.tile([128, NW, 512], BF16, tag="D_sb")
            ones = per.tile([1, 512], BF16, tag="ones")
            ident = per.tile([128, 128], BF16, tag="ident")
            sidx_sb = per.tile([128, NW * TPW * 8], I16, tag="sidx")
            wa2_sb = per.tile([128, 256], BF16, tag="wa2")

            nc.sync.dma_start(out=hT[:].rearrange("p c n -> p (c n)"), in_=h0T_d[:])
            nc.sync.dma_start(out=sidx_sb[:], in_=sidx_d[:])
            nc.vector.memset(ones[:], 1.0)
            nc.gpsimd.memset(ident[:], 0.0)
            from concourse.masks import make_identity
            make_identity(nc, ident[:], nomemset=True)

            for l in range(cfg.L):
                wb = wgt.tile([128, meta["wblob_cols"]], BF16, tag="wb")
                nc.sync.dma_start(out=wb[:], in_=wblob_d[l])
                nc.sync.dma_start(out=wa2_sb[:], in_=wa2_d[l])
                nc.sync.dma_start(out=D_sb[124:128, :, :].rearrange("p w d -> p (w d)"), in_=cin_d[l])
                # bf16 copy of h
                nc.vector.tensor_copy(out=hTb[:].rearrange("p c n -> p (c n)"),
                                      in_=hT[:].rearrange("p c n -> p (c n)"))

                # ---------- PROJ: P = h @ Wcat, node-major [124, 1024] per window
                for w in range(NW):
                    ns = slice(w * WN, (w + 1) * WN)
                    ps_d = psA.tile([124, 512], F32, tag="dc")
                    ps_s = psA.tile([124, 512], F32, tag="dc")
                    for kc in range(2):
                        lhs = hTb[:, kc, ns]
                        nc.tensor.matmul(ps_d[:], lhsT=lhs, rhs=wb[:, kc * 1024 : kc * 1024 + 512],
                                         start=(kc == 0), stop=(kc == 1))
                        nc.tensor.matmul(ps_s[:], lhsT=lhs, rhs=wb[:, kc * 1024 + 512 : (kc + 1) * 1024],
                                         start=(kc == 0), stop=(kc == 1))
                    nc.scalar.activation(out=D_sb[0:124, w, :], in_=ps_d[:], func=AF.Copy)
                    s_stage = wrk.tile([128, HT, 512], BF16, tag="pre")
                    nc.scalar.activation(out=s_stage[0:124, 0, :], in_=ps_s[:], func=AF.Copy)
                    nc.sync.dma_start(out=ag_in[w * WN : (w + 1) * WN, :], in_=s_stage[0:124, 0, :])

                # ---------- AllGather S table
                if "nocc" not in ABL:
                    nc.gpsimd.collective_compute(
                        "AllGather", OP.bypass, replica_groups=grp,
                        ins=[ag_in[:, :].opt()], outs=[s_full[:, :].opt()],
                    )

                # ---------- edge phase
                for w in range(NW):
                    ps_t = psT.tile([124, 257], F32, tag="t")
                    nc.vector.memset(ps_t[:], 0.0)
                    for hw in range([]. __len__() if "noedge" in ABL else 2):
                        t0 = hw * HT
                        sg = stp.tile([128, HT, 512], BF16, tag="sg")
                        nc.gpsimd.dma_gather(
                            out_ap=sg[:], in_ap=s_full[:, :],
                            idxs_ap=sidx_sb[:, (w * TPW + t0) * 8 : (w * TPW + t0 + HT) * 8],
                            num_idxs=HT * 128, num_idxs_reg=nidx_reg, elem_size=512,
                        )
                        qa = stp.tile([128, HT * 128], BF16, tag="qa")
                        nc.sync.dma_start(out=qa[:], in_=qat_d[:, (w * TPW + t0) * 128 : (w * TPW + t0 + HT) * 128])
                        qft = stp.tile([128, HT * 124], BF16, tag="qft")
                        nc.sync.dma_start(out=qft[:], in_=qf_d[:, (w * TPW + t0) * 124 : (w * TPW + t0 + HT) * 124])

                        pre = wrk.tile([128, HT, 512], BF16, tag="pre")
                        for t in range(HT):
                            ps_dc = psA.tile([128, 512], F32, tag="dc")
                            nc.tensor.matmul(ps_dc[:], lhsT=qa[:, t * 128 : (t + 1) * 128],
                                             rhs=D_sb[:, w, :], start=True, stop=True)
                            nc.scalar.activation(out=pre[:, t, :], in_=ps_dc[:], func=AF.Copy)
                        # pre += gathered src projections (TT add hits DVE 2x mode)
                        nc.vector.tensor_tensor(
                            out=pre[:].rearrange("p t d -> p (t d)"),
                            in0=pre[:].rearrange("p t d -> p (t d)"),
                            in1=sg[:].rearrange("p t d -> p (t d)"), op=OP.add)
                        # attention logits: sum over relu(pre_a) * wa2
                        # (sg is dead after the X1 add; reuse it as scratch)
                        tmpy = sg[:, :, 0:256]
                        nc.vector.scalar_tensor_tensor(
                            out=tmpy, in0=pre[:, :, 256:512], scalar=0.0,
                            in1=wa2_sb[:].rearrange("p (t d) -> p t d", t=1).to_broadcast([128, HT, 256]),
                            op0=OP.max, op1=OP.mult)
                        logit = sml.tile([128, HT], F32, tag="logit")
                        nc.vector.tensor_reduce(out=logit[:], in_=tmpy, axis=AX.X, op=OP.add)
                        contrib = wrk.tile([128, HT, 257], BF16, tag="contrib")
                        att = sml.tile([128, HT], F32, tag="att")
                        nc.scalar.activation(out=att[:], in_=logit[:],
                                             func=AF.Sigmoid, bias=meta["ba2s"][l], scale=1.0)
                        nc.vector.tensor_copy(out=contrib[:, :, 256:257], in_=att[:].rearrange("p (t o) -> p t o", o=1))
                        for t in range(HT):
                            nc.vector.tensor_scalar(out=contrib[:, t, 0:256], in0=pre[:, t, 0:256],
                                                    scalar1=att[:, t : t + 1], scalar2=0.0,
                                                    op0=OP.mult, op1=OP.max)
                        for t in range(HT):
                            nc.tensor.matmul(ps_t[:], lhsT=qft[:, t * 124 : (t + 1) * 124],
                                             rhs=contrib[:, t, :], start=False, stop=(hw == 1 and t == HT - 1),
                                             skip_group_check=True)
                    # ---- window tail: t -> transpose -> agg
                    t_sb = sml.tile([124, 257], BF16, tag="tsb")
                    nc.scalar.activation(out=t_sb[:], in_=ps_t[:], func=AF.Copy)
                    taugT = sml.tile([128, 2, 124], BF16, tag="taugT")
                    for fc in range(2):
                        ps_tr = psS.tile([128, 124], BF16, tag="s")
                        nc.tensor.transpose(out=ps_tr[:], in_=t_sb[:, fc * 128 : (fc + 1) * 128],
                                            identity=ident[0:124, 0:124])
                        nc.scalar.activation(out=taugT[:, fc, :], in_=ps_tr[:], func=AF.Copy)
                    ps_ta = psS.tile([1, 124], BF16, tag="s")
                    nc.tensor.transpose(out=ps_ta[:], in_=t_sb[:, 256:257], identity=ident[0:124, 0:124])
                    attsT = sml.tile([1, 124], BF16, tag="attsT")
                    nc.scalar.activation(out=attsT[:, :], in_=ps_ta[:], func=AF.Copy)
                    # agg.T = Wm2aug.T @ [t.T ; attsum]
                    for mc in range(2):
                        ps_ag = psS.tile([128, 124], F32, tag="s")
                        ms = slice(mc * 128, (mc + 1) * 128)
                        nc.tensor.matmul(ps_ag[:], lhsT=wb[:, 2048:2304][:, ms], rhs=taugT[:, 0, :], start=True, stop=False)
                        nc.tensor.matmul(ps_ag[:], lhsT=wb[:, 2304:2560][:, ms], rhs=taugT[:, 1, :], start=False, stop=False)
                        nc.tensor.matmul(ps_ag[:], lhsT=wb[0:1, 2560:2816][:, ms],
                                         rhs=attsT[:, :], start=False, stop=True)
                        nc.scalar.activation(out=aggT[:, mc, w * WN : (w + 1) * WN], in_=ps_ag[:], func=AF.Copy)

                # ---------- GRU (batched over node chunks)
                IH, HH = 2816, 5120
                NCH = -(-NL // 400)
                while NL % NCH:
                    NCH += 1
                CHW = NL // NCH
                assert CHW * NCH == NL and CHW <= 512

                def gemm_gate(ps, ms, rhs_a, rhs_h, a_only=False, h_only=False):
                    first = True
                    for woff, r2 in ((IH, rhs_a), (HH, rhs_h)):
                        if (a_only and woff == HH) or (h_only and woff == IH):
                            continue
                        nc.tensor.matmul(ps[:], lhsT=wb[:, woff : woff + 768][:, ms], rhs=r2[0],
                                         start=first, stop=False, skip_group_check=True)
                        nc.tensor.matmul(ps[:], lhsT=wb[:, woff + 768 : woff + 1536][:, ms], rhs=r2[1],
                                         start=False, stop=False, skip_group_check=True)
                        last = (woff == HH) or a_only or h_only
                        nc.tensor.matmul(ps[:], lhsT=wb[0:1, woff + 1536 : woff + 2304][:, ms], rhs=r2[2],
                                         start=False, stop=last, skip_group_check=True)
                        first = False

                for ch in range(NCH):
                    nsl = slice(ch * CHW, (ch + 1) * CHW)
                    rhs_a = (aggT[:, 0, nsl], aggT[:, 1, nsl], ones[:, 0:CHW])
                    rhs_h = (hTb[:, 0, nsl], hTb[:, 1, nsl], ones[:, 0:CHW])
                    rz_sb = sml.tile([128, 4, CHW], BF16, tag="rz")
                    for mc in range(4):  # r,z rows 0:512, both sides accumulated
                        ps = psA.tile([128, CHW], F32, tag="gru")
                        gemm_gate(ps, slice(mc * 128, (mc + 1) * 128), rhs_a, rhs_h)
                        nc.scalar.activation(out=rz_sb[:, mc, :], in_=ps[:], func=AF.Sigmoid)
                    for fc in range(2):  # n rows 512:768
                        ms = slice(512 + fc * 128, 512 + (fc + 1) * 128)
                        ps_in = psA.tile([128, CHW], F32, tag="gru")
                        gemm_gate(ps_in, ms, rhs_a, rhs_h, a_only=True)
                        ps_hn = psA.tile([128, CHW], F32, tag="gru")
                        gemm_gate(ps_hn, ms, rhs_a, rhs_h, h_only=True)
                        nn_sb = sml.tile([128, CHW], F32, tag="nn")
                        # nn = tanh(in + r*hn)
                        nc.vector.tensor_tensor(out=nn_sb[:], in0=rz_sb[:, fc, :], in1=ps_hn[:], op=OP.mult)
                        nc.vector.tensor_tensor(out=nn_sb[:], in0=nn_sb[:], in1=ps_in[:], op=OP.add)
                        nc.scalar.activation(out=nn_sb[:], in_=nn_sb[:], func=AF.Tanh)
                        # h' = nn + z*(h - nn)
                        hmn = sml.tile([128, CHW], F32, tag="hmn")
                        nc.vector.tensor_tensor(out=hmn[:], in0=hT[:, fc, nsl], in1=nn_sb[:], op=OP.subtract)
                        nc.vector.tensor_tensor(out=hmn[:], in0=hmn[:], in1=rz_sb[:, 2 + fc, :], op=OP.mult)
                        nc.vector.tensor_tensor(out=hT[:, fc, nsl], in0=hmn[:], in1=nn_sb[:], op=OP.add)

            if dbg_h is not None:
                nc.sync.dma_start(out=dbg_h[:, :], in_=hT[:].rearrange("p c n -> p (c n)"))
            # ================= tail =================
            tb = per.tile([128, meta["small_cols"]], BF16, tag="tail")
            nc.sync.dma_start(out=tb[:], in_=tail_d[:, 0 : meta["small_cols"]])
            rb = per.tile([1, meta["row_cols"]], BF16, tag="row")
            nc.sync.dma_start(out=rb[:], in_=row_d[:])
            pidx_sb = sml.tile([128, 16], I16, tag="pidx")
            nc.sync.dma_start(out=pidx_sb[:], in_=pidx_d[:])

            # node-major h (bf16) for prev-node gather, via PE transposes
            nc.vector.tensor_copy(out=hTb[:].rearrange("p c n -> p (c n)"),
                                  in_=hT[:].rearrange("p c n -> p (c n)"))
            hnm_sb = wgt.tile([124, 2, 128], BF16, tag="hnm")
            for w in range(NW):
                for fc in range(2):
                    ps_tr = psS.tile([128, 128], BF16, tag="s")
                    nc.tensor.transpose(out=ps_tr[0:124, :], in_=hTb[:, fc, w * WN : (w + 1) * WN], identity=ident[:])
                    nc.scalar.activation(out=hnm_sb[:, fc, :], in_=ps_tr[0:124, :], func=AF.Copy)
                nc.sync.dma_start(out=hnm_in[w * WN : (w + 1) * WN, :],
                                  in_=hnm_sb[:].rearrange("p c d -> p (c d)"))
            # local sums for hG
            hsum = sml.tile([128, 2], F32, tag="hsum")
            nc.vector.tensor_reduce(out=hsum[:], in_=hT[:], axis=AX.X, op=OP.add)
            nc.sync.dma_start(out=red_in[:, :], in_=hsum[:])
            if "nocc" not in ABL:
                nc.gpsimd.collective_compute("AllGather", OP.bypass, replica_groups=grp,
                                             ins=[hnm_in[:, :].opt()], outs=[hnm_full[:, :].opt()])
                nc.gpsimd.collective_compute("AllReduce", OP.add, replica_groups=grp,
                                             ins=[red_in[:, :].opt()], outs=[red_out[:, :].opt()])
            hG = sml.tile([128, 2], F32, tag="hG")
            nc.sync.dma_start(out=hG[:], in_=red_out[:, :])
            gp = wgt.tile([128, 2, 256], BF16, tag="gp")
            nc.gpsimd.dma_gather(out_ap=gp[:], in_ap=hnm_full[:, :], idxs_ap=pidx_sb[:],
                                 num_idxs=256, num_idxs_reg=nidx_reg2, elem_size=256)

            M2 = M + 1
            eiT = per.tile([128, 6, M2], BF16, tag="eiT")
            invN = 1.0 / cfg.N
            for fc in range(2):
                # hG rows (scaled by 1/N)
                nc.vector.tensor_scalar(out=eiT[:, fc, :], in0=hG[:, fc : fc + 1].to_broadcast([128, M2]),
                                        scalar1=invN, scalar2=None, op0=OP.mult)
                # t_emb rows: transpose gathered row M (target)
                ps_te = psS.tile([128, 1], BF16, tag="s")
                nc.tensor.transpose(out=ps_te[:], in_=gp[0:1, 1, fc * 128 : (fc + 1) * 128],
                                    identity=ident[0:1, 0:1])
                te = sml.tile([128, 1], BF16, tag="tecp")
                nc.scalar.activation(out=te[:], in_=ps_te[:], func=AF.Copy)
                nc.vector.tensor_scalar(out=eiT[:, 2 + fc, :], in0=te[:].to_broadcast([128, M2]),
                                        scalar1=1.0, scalar2=None, op0=OP.mult)
                # h_vj rows: transpose gathered rows 0:M
                ps_tv = psS.tile([128, M2], BF16, tag="s")
                nc.tensor.transpose(out=ps_tv[:, 0:M], in_=gp[0:M, 0, fc * 128 : (fc + 1) * 128],
                                    identity=ident[0:M, 0:M])
                nc.vector.memset(eiT[:, 4 + fc, :], 0.0)
                nc.scalar.activation(out=eiT[:, 4 + fc, 0:M], in_=ps_tv[:, 0:M], func=AF.Copy)

            def mlp_head(w1t, w1off, b1off, kchunks, outT):
                # outT [128, 2, M2] bf16 = relu(W1.T @ eiT + b1)
                for mc in range(2):
                    ps = psS.tile([128, M2], F32, tag="s")
                    for kc in range(kchunks):
                        nc.tensor.matmul(ps[:], lhsT=w1t[:, w1off + kc * 256 : w1off + kc * 256 + 256][:, mc * 128 : (mc + 1) * 128],
                                         rhs=eiT[:, kc, :], start=(kc == 0), stop=(kc == kchunks - 1))
                    nc.scalar.activation(out=outT[:, mc, :], in_=ps[:], func=AF.Relu,
                                         bias=tb[:, b1off + mc : b1off + mc + 1], scale=1.0)

            def softmax_free(x_sb, n_p, n_f):
                mx = sml.tile([n_p, 1], F32, tag="mx")
                nc.vector.tensor_reduce(out=mx[:], in_=x_sb[:], axis=AX.X, op=OP.max, negate=True)
                se = sml.tile([n_p, 1], F32, tag="se")
                nc.scalar.activation(out=x_sb[:], in_=x_sb[:], func=AF.Exp, bias=mx[:], scale=1.0, accum_out=se[:])
                nc.vector.reciprocal(out=se[:], in_=se[:])
                nc.vector.tensor_scalar(out=x_sb[:], in0=x_sb[:], scalar1=se[:], scalar2=None, op0=OP.mult)

            if dbg_ei is not None:
                nc.gpsimd.dma_start(out=dbg_ei[:, :], in_=eiT[:].rearrange("p c n -> p (c n)"))
            # ---- mix head
            mixT = sml.tile([128, 2, M2], BF16, tag="mixT")
            mlp_head(tb, toffs["mW1"], toffs["mb1"], 6, mixT)
            mixl = sml.tile([M2, K], F32, tag="mixl")
            ps_ml = psS.tile([M2, K], F32, tag="s")
            for mc in range(2):
                nc.tensor.matmul(ps_ml[:], lhsT=mixT[:, mc, :], rhs=tb[:, toffs["mW2"] + mc * K : toffs["mW2"] + (mc + 1) * K],
                                 start=(mc == 0), stop=False, skip_group_check=True)
            nc.tensor.matmul(ps_ml[:], lhsT=ones[:, 0:M2], rhs=rb[:, roffs["mb2"] : roffs["mb2"] + K],
                             start=False, stop=True, skip_group_check=True)
            nc.scalar.activation(out=mixl[:], in_=ps_ml[:], func=AF.Copy)
            softmax_free(mixl, M2, K)

            if dbg_mix is not None:
                nc.sync.dma_start(out=dbg_mix[:, :], in_=mixl[:])
            # ---- edge heads
            acc = sml.tile([M2, ET], F32, tag="acc")
            nc.vector.memset(acc[:], 0.0)
            hkT = sml.tile([128, 2, M2], BF16, tag="hkT")
            for k in range(K):
                ekw = wgt.tile([128, 6 * 256], BF16, tag="ekw")
                nc.sync.dma_start(out=ekw[:], in_=tail_d[:, toffs["eW1"] + k * 6 * 256 : toffs["eW1"] + (k + 1) * 6 * 256])
                mlp_head(ekw, 0, toffs["eb1"] + k * 2, 6, hkT)
                ps_el = psS.tile([M2, ET], F32, tag="s")
                for mc in range(2):
                    nc.tensor.matmul(ps_el[:], lhsT=hkT[:, mc, :],
                                     rhs=tb[:, toffs["eW2"] + (k * 2 + mc) * ET : toffs["eW2"] + (k * 2 + mc + 1) * ET],
                                     start=(mc == 0), stop=False, skip_group_check=True)
                nc.tensor.matmul(ps_el[:], lhsT=ones[:, 0:M2],
                                 rhs=rb[:, roffs["eb2"] + k * ET : roffs["eb2"] + (k + 1) * ET],
                                 start=False, stop=True, skip_group_check=True)
                ekl = sml.tile([M2, ET], F32, tag="ekl")
                nc.scalar.activation(out=ekl[:], in_=ps_el[:], func=AF.Copy)
                softmax_free(ekl, M2, ET)
                nc.vector.scalar_tensor_tensor(out=acc[:], in0=ekl[:], scalar=mixl[:, k : k + 1],
                                               in1=acc[:], op0=OP.mult, op1=OP.add)
            nc.sync.dma_start(out=eprob_d[:, :], in_=acc[0:M, :])

            # ---- node head
            niT = sml.tile([128, 4], BF16, tag="niT")
            for fc in range(2):
                nc.vector.tensor_scalar(out=niT[:, fc : fc + 1], in0=hG[:, fc : fc + 1],
                                        scalar1=invN, scalar2=None, op0=OP.mult)
                nc.vector.tensor_copy(out=niT[:, 2 + fc : 3 + fc], in_=eiT[:, 2 + fc, 0:1])
            nhT = sml.tile([128, 2], BF16, tag="nhT")
            for mc in range(2):
                ps = psS.tile([128, 1], F32, tag="s")
                for kc in range(4):
                    nc.tensor.matmul(ps[:], lhsT=tb[:, toffs["nW1"] + kc * 256 : toffs["nW1"] + kc * 256 + 256][:, mc * 128 : (mc + 1) * 128],
                                     rhs=niT[:, kc : kc + 1], start=(kc == 0), stop=(kc == 3))
                nc.scalar.activation(out=nhT[:, mc : mc + 1], in_=ps[:], func=AF.Relu,
                                     bias=tb[:, toffs["nb1"] + mc : toffs["nb1"] + mc + 1], scale=1.0)
            ps_nl = psS.tile([1, NT], F32, tag="s")
            for mc in range(2):
                nc.tensor.matmul(ps_nl[:], lhsT=nhT[:, mc : mc + 1],
                                 rhs=tb[:, toffs["nW2"] + mc * NT : toffs["nW2"] + (mc + 1) * NT],
                                 start=(mc == 0), stop=False, skip_group_check=True)
            nc.tensor.matmul(ps_nl[:], lhsT=ones[:, 0:1], rhs=rb[:, roffs["nb2"] : roffs["nb2"] + NT],
                             start=False, stop=True, skip_group_check=True)
            nl_sb = sml.tile([1, NT], F32, tag="nlsb")
            nc.scalar.activation(out=nl_sb[:], in_=ps_nl[:], func=AF.Copy)
            softmax_free(nl_sb, 1, NT)
            nc.sync.dma_start(out=nprob_d[:, :], in_=nl_sb[:])

    nc.compile()
    return nc


# --------------------------------------------------------------------------
_LAST_RESULTS = {}


def _run_spmd_timed(nc, in_maps, n_cores, reps=3):
    """Mirror bass2jax.run_bass_via_pjrt's multi-core path, but jit once,
    device_put inputs once, and wall-clock steady-state executions."""
    import time
    import jax
    from jax.sharding import Mesh, PartitionSpec, NamedSharding
    from jax.experimental.shard_map import shard_map
    import concourse.mybir as mybir_
    from concourse import bass2jax

    bass2jax.install_neuronx_cc_hook()
    partition_name = nc.partition_id_tensor.name if nc.partition_id_tensor else None
    in_names, out_names, out_avals, zero_shapes = [], [], [], []
    for alloc in nc.m.functions[0].allocations:
        if not isinstance(alloc, mybir_.MemoryLocationSet):
            continue
        name = alloc.memorylocations[0].name
        if alloc.kind == "ExternalInput":
            if name != partition_name:
                in_names.append(name)
        elif alloc.kind == "ExternalOutput":
            out_names.append(name)
            shape = tuple(alloc.tensor_shape)
            dtype = mybir_.dt.np(alloc.dtype)
            out_avals.append(jax.core.ShapedArray(shape, dtype))
            zero_shapes.append((shape, dtype))
    n_params = len(in_names)
    n_outs = len(out_avals)
    all_names = list(in_names) + out_names + ([partition_name] if partition_name else [])
    donate = tuple(range(n_params, n_params + n_outs))

    def _body(*args):
        operands = list(args)
        if partition_name is not None:
            operands.append(bass2jax.partition_id_tensor())
        return tuple(bass2jax._bass_exec_p.bind(
            *operands, out_avals=tuple(out_avals), in_names=tuple(all_names),
            out_names=tuple(out_names), lowering_input_output_aliases=(),
            sim_require_finite=True, sim_require_nnan=True, nc=nc))

    devices = jax.devices()[:n_cores]
    mesh = Mesh(np.asarray(devices), ("core",))
    spec = PartitionSpec("core")
    sharded = jax.jit(
        shard_map(_body, mesh=mesh, in_specs=(spec,) * (n_params + n_outs),
                  out_specs=(spec,) * n_outs, check_rep=False),
        donate_argnums=donate, keep_unused=True)
    concat_in = [
        np.concatenate([np.asarray(in_maps[c][nm]) for c in range(n_cores)], axis=0)
        for nm in in_names
    ]
    sh = NamedSharding(mesh, spec)
    dev_in = [jax.device_put(a, sh) for a in concat_in]

    def zeros():
        return [jax.device_put(np.zeros((n_cores * s[0], *s[1:]), d), sh)
                for s, d in zero_shapes]

    outs = sharded(*dev_in, *zeros())
    jax.block_until_ready(outs)
    times = []
    for _ in range(reps):
        z = zeros()
        jax.block_until_ready(z)
        t0 = time.perf_counter()
        outs = sharded(*dev_in, *z)
        jax.block_until_ready(outs)
        times.append(time.perf_counter() - t0)
    results = [
        {nm: np.asarray(outs[i]).reshape(n_cores, *zero_shapes[i][0])[c]
         for i, nm in enumerate(out_names)}
        for c in range(n_cores)
    ]
    return results, min(times)


def kernel(**inputs):
    from concourse.bass_utils import run_bass_kernel_spmd

    C = 8
    in_maps, meta = preprocess(inputs, C=C)
    nc = build(meta)
    if os.environ.get("BASSGNN_TIME", "0") == "1":
        results, secs = _run_spmd_timed(nc, in_maps, C)
        _LAST_RESULTS["exec_time_ns"] = secs * 1e9
        node_probs = np.asarray(results[0]["node_probs"]).reshape(-1)
        edge_probs = np.asarray(results[0]["edge_probs"])
        return node_probs, edge_probs
    trace = os.environ.get("BASSGNN_TRACE", "0") == "1"
    res = run_bass_kernel_spmd(nc, in_maps, core_ids=list(range(C)), trace=trace)
    _LAST_RESULTS["res"] = res
    node_probs = np.asarray(res.results[0]["node_probs"]).reshape(-1)
    edge_probs = np.asarray(res.results[0]["edge_probs"])
    return node_probs, edge_probs


# revision 28
# speedup vs baseline: 1.2206x; 1.0930x over previous
"""Distributed Trainium2 Bass kernel for the DenoisingNetwork GNN.

Strategy (8-way node/edge sharding, graph-parallel):
  * Nodes are assigned to 8 cores x NW windows of up to 124 nodes each,
    balanced so every (core, window) slot has nearly equal edge counts
    (LPT bin packing on destination degree).  Edges live with their dst.
  * Key algebra: since attention is a per-edge scalar,
        segment_sum(att * (relu(pre) @ Wm2)) = segment_sum(att*relu(pre)) @ Wm2
    so the only E-scale work is elementwise + a one-hot segment matmul.
  * Per-edge pre-activations decompose into node projections:
        pre = [h@W_i](dst) + [h@W_j](src) + [eemb@W_e + b](attr)
    The src part is gathered from an AllGather'ed per-layer table via
    dma_gather; the dst+attr parts are broadcast per-edge with a one-hot
    matmul on the TensorEngine (PE emits D[dst]+C[attr] straight to PSUM).
  * GRU + projections are node-level feature-major matmuls; the tail
    (readout heads) is replicated on every core after tiny AllReduces.

kernel(**inputs) takes the full-size numpy inputs, shards on host,
runs the SPMD Bass kernel on 8 NeuronCores, and returns
(node_probs [NT], edge_probs [M, ET]) like the reference.
"""

import heapq
import os
import sys

sys.path.insert(0, "/opt/trn_rl_repo")

import numpy as np
import ml_dtypes

import concourse.bass as bass
import concourse.bacc as bacc
import concourse.mybir as mybir
import concourse.tile as tile
from concourse import library_config
from concourse.vector_clock import ScopedClock

BF16 = mybir.dt.bfloat16
F32 = mybir.dt.float32
I16 = mybir.dt.int16
AF = mybir.ActivationFunctionType
OP = mybir.AluOpType
AX = mybir.AxisListType

nbf = ml_dtypes.bfloat16


# --------------------------------------------------------------------------
# Walrus in this container rejects CTRL instructions with >2 sync waits;
# Tile's tail drain aggregates one wait per active proc.  Chunk them.
class TileContextChunkedDrain(tile.TileContext):
    def _drain_and_barrier(self, tick_clock, wait_clock):
        probe = self.nc.sync.drain()
        wait_clock.add_sem_waits(
            probe.ins, ScopedClock({None: tick_clock.global_clock})
        )
        si = probe.ins.sync_info
        waits = list(si.on_wait or []) if si is not None else []
        if len(waits) > 1:
            si.on_wait = waits[:1]
            for w in waits[1:]:
                d = self.nc.sync.drain()
                dsi = d.ins.sync_info
                if dsi is None:
                    d.ins.sync_info = type(si)(on_wait=[w], on_update=[])
                else:
                    dsi.on_wait = [w]
        self.nc.all_engine_barrier()
        assert self.sems is not None
        popped = self.nc._tile_sem_poison_stack.pop()
        assert popped is self._sem_poison
        self.nc.clear_and_free_semaphores(list(self.sems.allocated().values()))
        self.nc.all_engine_barrier()


# --------------------------------------------------------------------------
class Cfg:
    def __init__(self, N, E, C, L, NT, ET, K, M, TPW, NW):
        self.N = N            # total nodes
        self.E = E            # total edges
        self.C = C            # cores
        self.L = L            # message passing layers
        self.NT = NT          # node types (+mask)
        self.ET = ET          # edge types (+mask)
        self.K = K            # mixture components
        self.M = M            # previous nodes
        self.H = 256
        self.WN = 124         # nodes per window
        self.NW = NW          # windows per core
        self.NL = NW * self.WN  # node slots per core
        self.R = C * self.NL    # global table rows
        self.TPW = TPW        # 128-edge tiles per window (even)
        self.EW = TPW * 128   # edge slots per window
        assert TPW % 2 == 0
        self.HT = TPW // 2    # tiles per half-window


def _lpt_assign(deg, C, NW, WN):
    """Assign nodes to C*NW slots (cap WN nodes each), balancing edge counts."""
    nslots = C * NW
    order = np.argsort(-deg, kind="stable")
    heap = [(0, 0, s) for s in range(nslots)]  # (edges, nodes, slot)
    heapq.heapify(heap)
    slot_of = np.empty(len(deg), np.int32)
    pos_of = np.empty(len(deg), np.int32)
    nodes_in = np.zeros(nslots, np.int64)
    spill = []
    for g in order:
        while True:
            e, n, s = heapq.heappop(heap)
            if n < WN:
                break
            spill.append((e, n, s))
        slot_of[g] = s
        pos_of[g] = n
        nodes_in[s] = n + 1
        heapq.heappush(heap, (e + int(deg[g]), n + 1, s))
        for item in spill:
            heapq.heappush(heap, item)
        spill.clear()
    return slot_of, pos_of


def preprocess(inputs, C=8, L=None):
    x_nodes = np.asarray(inputs["x_nodes"])
    edge_index = np.asarray(inputs["edge_index"])
    edge_attr = np.asarray(inputs["edge_attr"])
    target = int(np.asarray(inputs["target_node_idx"]))
    prev = np.asarray(inputs["previous_nodes"])
    node_emb = np.asarray(inputs["node_emb"], np.float32)
    eemb_t = np.asarray(inputs["edge_emb_table"], np.float32)
    mp = inputs["mp_params"]
    node_pred = inputs["node_pred"]
    edge_pred = inputs["edge_pred"]
    mix_pred = inputs["mix_pred"]

    N = x_nodes.shape[0]
    E = edge_index.shape[1]
    if L is None:
        L = len(mp)
    NT = node_emb.shape[0]
    ET = eemb_t.shape[0]
    K = np.asarray(edge_pred["W1"]).shape[0]
    M = prev.shape[0]
    H = 256
    WN = 124
    NW = -(-N // (C * WN))

    src, dst = edge_index[0], edge_index[1]
    deg = np.bincount(dst, minlength=N)
    slot_of, pos_of = _lpt_assign(deg, C, NW, WN)
    edges_per_slot = np.bincount(slot_of[dst], minlength=C * NW)
    TPW = max(2, int(-(-edges_per_slot.max() // 128)))
    if TPW % 2:
        TPW += 1
    cfg = Cfg(N, E, C, L, NT, ET, K, M, TPW, NW)
    NL, R, EW = cfg.NL, cfg.R, cfg.EW

    core_of = slot_of // NW
    win_of = slot_of % NW
    newrow = core_of.astype(np.int64) * NL + win_of * WN + pos_of

    # ---- per-edge placement
    e_core = core_of[dst]
    e_win = win_of[dst]
    e_pos = pos_of[dst]
    e_src_row = newrow[src]
    # sort edges by (core, window, pos) and lay out into padded slots
    eorder = np.lexsort((e_pos, e_win, e_core))
    sC, sW, sP = e_core[eorder], e_win[eorder], e_pos[eorder]
    sSrc, sAttr = e_src_row[eorder], edge_attr[eorder]
    # slot index within (core, win): cumulative count
    swkey = sC.astype(np.int64) * NW + sW
    slot_start = np.zeros(C * NW + 1, np.int64)
    np.cumsum(np.bincount(swkey, minlength=C * NW), out=slot_start[1:])
    j_in_slot = np.arange(E) - slot_start[swkey]

    qat = np.zeros((C, 128, NW * TPW * 128), np.float32)
    qf = np.zeros((C, 128, NW * TPW * 124), np.float32)
    sidx = np.zeros((C, NW * EW), np.int64)
    col = sW * EW + j_in_slot
    tno = j_in_slot >> 7
    rno = j_in_slot & 127
    qat[sC, sP, col] = 1.0
    qat[sC, 124 + sAttr, col] = 1.0
    qf[sC, rno, (sW * TPW + tno) * 124 + sP] = 1.0
    sidx[sC, col] = sSrc

    # wrapped int16 index layout for dma_gather: [128, n//16] replicated x8
    def wrap_idx(flat):
        n = flat.shape[0]
        w = flat.reshape(n // 16, 16).T
        return np.tile(w, (8, 1)).astype(np.int16)

    # ---- initial node features, feature-major per core
    h0 = node_emb[x_nodes]  # [N, H]
    h0_all = np.zeros((R, H), np.float32)
    h0_all[newrow] = h0

    # ---- weights
    def b16(x):
        return np.asarray(x, np.float32).astype(nbf)

    WBLOB_COLS = 2048 + 768 + 2 * 2304
    wblob = np.zeros((L, 128, WBLOB_COLS), np.float32)
    c_in = np.zeros((L, 4, NW * 512), np.float32)
    ba2s = []
    for l in range(L):
        p = {k: np.asarray(v, np.float32) for k, v in mp[l].items()}
        wcat = np.concatenate(
            [p["Wm1"][0:H], p["Wa1"][0:H], p["Wm1"][H : 2 * H], p["Wa1"][H : 2 * H]],
            axis=1,
        )  # [256, 1024] cols [D_m|D_a|S_m|S_a]
        wblob[l, :, 0:1024] = wcat[0:128]
        wblob[l, :, 1024:2048] = wcat[128:256]
        w2a = np.concatenate([p["Wm2"], p["bm2"][None, :]], axis=0)  # [257, 256]
        wblob[l, :, 2048:2304] = w2a[0:128]
        wblob[l, :, 2304:2560] = w2a[128:256]
        wblob[l, 0, 2560:2816] = w2a[256]
        for off, (wname, bname) in ((2816, ("Wih", "bih")), (5120, ("Whh", "bhh"))):
            wa = np.concatenate([p[wname], p[bname][None, :]], axis=0)  # [257, 768]
            wblob[l, :, off : off + 768] = wa[0:128]
            wblob[l, :, off + 768 : off + 1536] = wa[128:256]
            wblob[l, 0, off + 1536 : off + 2304] = wa[256]
        cm = eemb_t @ p["Wm1"][2 * H : 3 * H] + p["bm1"][None, :]
        ca = eemb_t @ p["Wa1"][2 * H : 3 * H] + p["ba1"][None, :]
        cful = np.concatenate([cm, ca], axis=1)  # [4, 512]
        c_in[l] = np.tile(cful[:, None, :], (1, NW, 1)).reshape(4, NW * 512)
        ba2s.append(float(p["ba2"].reshape(-1)[0]))
    wa2_tile = np.tile(
        np.asarray(mp[0]["Wa2"], np.float32).reshape(1, H), (128, 1)
    )  # per-layer below
    wa2_all = np.stack(
        [np.tile(np.asarray(mp[l]["Wa2"], np.float32).reshape(1, H), (128, 1)) for l in range(L)]
    )  # [L, 128, 256]

    # ---- tail weights blob
    npd = {k: np.asarray(v, np.float32) for k, v in node_pred.items()}
    epd = {k: np.asarray(v, np.float32) for k, v in edge_pred.items()}
    mpd = {k: np.asarray(v, np.float32) for k, v in mix_pred.items()}
    cols = []
    offs = {}

    def add_block(name, arr128):
        offs[name] = sum(a.shape[1] for a in cols)
        cols.append(arr128.astype(np.float32))

    def chunk_lhsT(w):  # [Kdim, Mdim] -> [128, (Kdim//128)*Mdim]
        kd, md = w.shape
        assert kd % 128 == 0
        return np.concatenate([w[i * 128 : (i + 1) * 128] for i in range(kd // 128)], axis=1)

    add_block("nW1", chunk_lhsT(npd["W1"]))          # 4*256
    add_block("nb1", npd["b1"].reshape(2, 128).T)     # [128,2]
    add_block("nW2", chunk_lhsT(npd["W2"]))           # 2*10
    add_block("mW1", chunk_lhsT(mpd["W1"]))           # 6*256
    add_block("mb1", mpd["b1"].reshape(2, 128).T)
    add_block("mW2", chunk_lhsT(mpd["W2"]))           # 2*20
    add_block("eb1", np.concatenate([epd["b1"][k].reshape(2, 128).T for k in range(K)], axis=1))
    add_block("eW2", np.concatenate([chunk_lhsT(epd["W2"][k]) for k in range(K)], axis=1))
    add_block("eW1", np.concatenate([chunk_lhsT(epd["W1"][k]) for k in range(K)], axis=1))
    tailblob = np.concatenate(cols, axis=1)
    small_cols = offs["eW1"]
    rowblob = np.zeros((1, 16 + 32 + 4 * K), np.float32)
    rowblob[0, 0:NT] = npd["b2"]
    rowblob[0, 16 : 16 + K] = mpd["b2"]
    rowblob[0, 48 : 48 + 4 * K] = epd["b2"].reshape(-1)
    roffs = {"nb2": 0, "mb2": 16, "eb2": 48}

    # ---- prev/target gather rows (target at slot 128 -> tile 1 partition 0)
    # Per-core LOCAL indices into hnm_in; non-owned rows hit the zero row at NL
    # and the cross-core merge is a tiny AllReduce (zeros add exactly).
    prow, trow = newrow[prev], newrow[target]
    pidx_all = []
    for c in range(C):
        pidx = np.full(256, NL, np.int64)
        own = (prow // NL) == c
        pidx[:M][own] = prow[own] % NL
        if trow // NL == c:
            pidx[128] = trow % NL
        pidx_all.append(pidx)

    meta = dict(cfg=cfg, ba2s=ba2s, toffs=offs, roffs=roffs,
                tail_cols=tailblob.shape[1], row_cols=rowblob.shape[1],
                small_cols=small_cols, wblob_cols=WBLOB_COLS)

    in_maps = []
    for c in range(C):
        blk = h0_all[c * NL : (c + 1) * NL]  # [NL, 256]
        h0T = np.concatenate([blk.T[0:128], blk.T[128:256]], axis=1)  # [128, 2*NL]
        m = {
            "h0T": np.ascontiguousarray(h0T, np.float32),
            "qat": np.ascontiguousarray(qat[c]).astype(nbf),
            "qf": np.ascontiguousarray(qf[c]).astype(nbf),
            "sidx": np.ascontiguousarray(
                np.concatenate([wrap_idx(sidx[c, w * EW : (w + 1) * EW]) for w in range(NW)], axis=1)
            ),
            "pidx": wrap_idx(pidx_all[c]),
            "wblob": wblob.astype(nbf),
            "c_in": c_in.astype(nbf),
            "wa2t": wa2_all.astype(nbf),
            "tailblob": tailblob.astype(nbf),
            "rowblob": rowblob.astype(nbf),
        }
        in_maps.append(m)
    return in_maps, meta


# --------------------------------------------------------------------------
def build(meta):
    cfg = meta["cfg"]
    C, L, NW, NL, WN, TPW, HT, EW = (
        cfg.C, cfg.L, cfg.NW, cfg.NL, cfg.WN, cfg.TPW, cfg.HT, cfg.EW,
    )
    R, H, K, M, NT, ET = cfg.R, cfg.H, cfg.K, cfg.M, cfg.NT, cfg.ET
    HT = cfg.HT
    ABL = os.environ.get("BASSGNN_ABLATE", "")
    toffs, roffs = meta["toffs"], meta["roffs"]
    grp = [list(range(C))]

    nc = bacc.Bacc("TRN2", target_bir_lowering=False, debug=False, num_devices=C,
                   dynamic_dma_scratch_size=65536)
    # ---------- I/O
    h0T_d = nc.dram_tensor("h0T", [128, 2 * NL], F32, kind="ExternalInput")
    qat_d = nc.dram_tensor("qat", [128, NW * TPW * 128], BF16, kind="ExternalInput")
    qf_d = nc.dram_tensor("qf", [128, NW * TPW * 124], BF16, kind="ExternalInput")
    sidx_d = nc.dram_tensor("sidx", [128, NW * TPW * 8], I16, kind="ExternalInput")
    pidx_d = nc.dram_tensor("pidx", [128, 16], I16, kind="ExternalInput")
    wblob_d = nc.dram_tensor("wblob", [L, 128, meta["wblob_cols"]], BF16, kind="ExternalInput")
    cin_d = nc.dram_tensor("c_in", [L, 4, NW * 512], BF16, kind="ExternalInput")
    wa2_d = nc.dram_tensor("wa2t", [L, 128, 256], BF16, kind="ExternalInput")
    tail_d = nc.dram_tensor("tailblob", [128, meta["tail_cols"]], BF16, kind="ExternalInput")
    row_d = nc.dram_tensor("rowblob", [1, meta["row_cols"]], BF16, kind="ExternalInput")
    nprob_d = nc.dram_tensor("node_probs", [1, NT], F32, kind="ExternalOutput")
    dbg_h = nc.dram_tensor("dbg_h", [128, 2 * NL], F32, kind="ExternalOutput") if os.environ.get("BASSGNN_DBG") else None
    dbg_ei = nc.dram_tensor("dbg_ei", [128, 6 * (M + 1)], F32, kind="ExternalOutput") if os.environ.get("BASSGNN_DBG") else None
    dbg_mix = nc.dram_tensor("dbg_mix", [M + 1, K], F32, kind="ExternalOutput") if os.environ.get("BASSGNN_DBG") else None
    eprob_d = nc.dram_tensor("edge_probs", [M, ET], F32, kind="ExternalOutput")

    # internal DRAM (Shared addr space only supported for >4-core groups)
    shared = "Shared" if C > 4 else "Local"
    ag_in = nc.dram_tensor("ag_in", [NL, 512], BF16)
    s_full = nc.dram_tensor("s_full", [R, 512], BF16, addr_space=shared)
    hnm_in = nc.dram_tensor("hnm_in", [NL + 16, 256], BF16)
    red2_in = nc.dram_tensor("red2_in", [128, 512], BF16)
    red2_out = nc.dram_tensor("red2_out", [128, 512], BF16, addr_space=shared)
    red_in = nc.dram_tensor("red_in", [128, 2], F32)
    red_out = nc.dram_tensor("red_out", [128, 2], F32, addr_space=shared)

    with TileContextChunkedDrain(nc) as tc:
        with (
            tc.tile_pool(name="per", bufs=1) as per,          # persistent
            tc.tile_pool(name="wgt", bufs=1) as wgt,          # per-layer weights
            tc.tile_pool(name="str", bufs=2) as stp,          # streamed Q / gathers
            tc.tile_pool(name="wrk", bufs=2) as wrk,          # edge work tensors
            tc.tile_pool(name="sml", bufs=2) as sml,          # small tiles
            tc.tile_pool(name="psA", bufs=2, space="PSUM") as psA,   # dc / proj
            tc.tile_pool(name="psT", bufs=2, space="PSUM") as psT,   # t window accum
            tc.tile_pool(name="psS", bufs=2, space="PSUM") as psS,   # small psums
        ):
            nidx_reg = nc.gpsimd.alloc_register("nidx")
            nc.gpsimd.reg_mov(nidx_reg, HT * 128)
            nidx_reg2 = nc.gpsimd.alloc_register("nidx2")
            nc.gpsimd.reg_mov(nidx_reg2, 256)

            hT = per.tile([128, 2, NL], F32, tag="hT")
            hTb = per.tile([128, 2, NL], BF16, tag="hTb")
            aggT = per.tile([128, 2, NL], BF16, tag="aggT")
            D_sb = per.tile([128, NW, 512], BF16, tag="D_sb")
            ones = per.tile([1, 512], BF16, tag="ones")
            ident = per.tile([128, 128], BF16, tag="ident")
            sidx_sb = per.tile([128, NW * TPW * 8], I16, tag="sidx")
            wa2_sb = per.tile([128, 256], BF16, tag="wa2")

            nc.sync.dma_start(out=hT[:].rearrange("p c n -> p (c n)"), in_=h0T_d[:])
            nc.sync.dma_start(out=sidx_sb[:], in_=sidx_d[:])
            nc.vector.memset(ones[:], 1.0)
            nc.gpsimd.memset(ident[:], 0.0)
            from concourse.masks import make_identity
            make_identity(nc, ident[:], nomemset=True)

            for l in range(cfg.L):
                wb = wgt.tile([128, meta["wblob_cols"]], BF16, tag="wb")
                nc.sync.dma_start(out=wb[:], in_=wblob_d[l])
                nc.sync.dma_start(out=wa2_sb[:], in_=wa2_d[l])
                nc.sync.dma_start(out=D_sb[124:128, :, :].rearrange("p w d -> p (w d)"), in_=cin_d[l])
                # bf16 copy of h
                nc.vector.tensor_copy(out=hTb[:].rearrange("p c n -> p (c n)"),
                                      in_=hT[:].rearrange("p c n -> p (c n)"))

                # ---------- PROJ: P = h @ Wcat, node-major [124, 1024] per window
                for w in range(NW):
                    ns = slice(w * WN, (w + 1) * WN)
                    ps_d = psA.tile([124, 512], F32, tag="dc")
                    ps_s = psA.tile([124, 512], F32, tag="dc")
                    for kc in range(2):
                        lhs = hTb[:, kc, ns]
                        nc.tensor.matmul(ps_d[:], lhsT=lhs, rhs=wb[:, kc * 1024 : kc * 1024 + 512],
                                         start=(kc == 0), stop=(kc == 1))
                        nc.tensor.matmul(ps_s[:], lhsT=lhs, rhs=wb[:, kc * 1024 + 512 : (kc + 1) * 1024],
                                         start=(kc == 0), stop=(kc == 1))
                    nc.scalar.activation(out=D_sb[0:124, w, :], in_=ps_d[:], func=AF.Copy)
                    s_stage = wrk.tile([128, HT, 512], BF16, tag="pre")
                    nc.scalar.activation(out=s_stage[0:124, 0, :], in_=ps_s[:], func=AF.Copy)
                    nc.sync.dma_start(out=ag_in[w * WN : (w + 1) * WN, :], in_=s_stage[0:124, 0, :])

                # ---------- AllGather S table
                if "nocc" not in ABL:
                    nc.gpsimd.collective_compute(
                        "AllGather", OP.bypass, replica_groups=grp,
                        ins=[ag_in[:, :].opt()], outs=[s_full[:, :].opt()],
                    )

                # ---------- edge phase
                for w in range(NW):
                    ps_t = psT.tile([124, 257], F32, tag="t")
                    nc.vector.memset(ps_t[:], 0.0)
                    for hw in range([]. __len__() if "noedge" in ABL else 2):
                        t0 = hw * HT
                        sg = stp.tile([128, HT, 512], BF16, tag="sg")
                        nc.gpsimd.dma_gather(
                            out_ap=sg[:], in_ap=s_full[:, :],
                            idxs_ap=sidx_sb[:, (w * TPW + t0) * 8 : (w * TPW + t0 + HT) * 8],
                            num_idxs=HT * 128, num_idxs_reg=nidx_reg, elem_size=512,
                        )
                        qa = stp.tile([128, HT * 128], BF16, tag="qa")
                        nc.sync.dma_start(out=qa[:], in_=qat_d[:, (w * TPW + t0) * 128 : (w * TPW + t0 + HT) * 128])
                        qft = stp.tile([128, HT * 124], BF16, tag="qft")
                        nc.sync.dma_start(out=qft[:], in_=qf_d[:, (w * TPW + t0) * 124 : (w * TPW + t0 + HT) * 124])

                        pre = wrk.tile([128, HT, 512], BF16, tag="pre")
                        for t in range(HT):
                            ps_dc = psA.tile([128, 512], F32, tag="dc")
                            nc.tensor.matmul(ps_dc[:], lhsT=qa[:, t * 128 : (t + 1) * 128],
                                             rhs=D_sb[:, w, :], start=True, stop=True)
                            nc.scalar.activation(out=pre[:, t, :], in_=ps_dc[:], func=AF.Copy)
                        # pre += gathered src projections (TT add hits DVE 2x mode)
                        nc.vector.tensor_tensor(
                            out=pre[:].rearrange("p t d -> p (t d)"),
                            in0=pre[:].rearrange("p t d -> p (t d)"),
                            in1=sg[:].rearrange("p t d -> p (t d)"), op=OP.add)
                        # attention logits: sum over relu(pre_a) * wa2
                        # (sg is dead after the X1 add; reuse it as scratch)
                        tmpy = sg[:, :, 0:256]
                        nc.vector.scalar_tensor_tensor(
                            out=tmpy, in0=pre[:, :, 256:512], scalar=0.0,
                            in1=wa2_sb[:].rearrange("p (t d) -> p t d", t=1).to_broadcast([128, HT, 256]),
                            op0=OP.max, op1=OP.mult)
                        logit = sml.tile([128, HT], F32, tag="logit")
                        nc.vector.tensor_reduce(out=logit[:], in_=tmpy, axis=AX.X, op=OP.add)
                        contrib = wrk.tile([128, HT, 257], BF16, tag="contrib")
                        att = sml.tile([128, HT], F32, tag="att")
                        nc.scalar.activation(out=att[:], in_=logit[:],
                                             func=AF.Sigmoid, bias=meta["ba2s"][l], scale=1.0)
                        nc.vector.tensor_copy(out=contrib[:, :, 256:257], in_=att[:].rearrange("p (t o) -> p t o", o=1))
                        for t in range(HT):
                            nc.vector.tensor_scalar(out=contrib[:, t, 0:256], in0=pre[:, t, 0:256],
                                                    scalar1=att[:, t : t + 1], scalar2=0.0,
                                                    op0=OP.mult, op1=OP.max)
                        for t in range(HT):
                            nc.tensor.matmul(ps_t[:], lhsT=qft[:, t * 124 : (t + 1) * 124],
                                             rhs=contrib[:, t, :], start=False, stop=(hw == 1 and t == HT - 1),
                                             skip_group_check=True)
                    # ---- window tail: t -> transpose -> agg
                    t_sb = sml.tile([124, 257], BF16, tag="tsb")
                    nc.scalar.activation(out=t_sb[:], in_=ps_t[:], func=AF.Copy)
                    taugT = sml.tile([128, 2, 124], BF16, tag="taugT")
                    for fc in range(2):
                        ps_tr = psS.tile([128, 124], BF16, tag="s")
                        nc.tensor.transpose(out=ps_tr[:], in_=t_sb[:, fc * 128 : (fc + 1) * 128],
                                            identity=ident[0:124, 0:124])
                        nc.scalar.activation(out=taugT[:, fc, :], in_=ps_tr[:], func=AF.Copy)
                    ps_ta = psS.tile([1, 124], BF16, tag="s")
                    nc.tensor.transpose(out=ps_ta[:], in_=t_sb[:, 256:257], identity=ident[0:124, 0:124])
                    attsT = sml.tile([1, 124], BF16, tag="attsT")
                    nc.scalar.activation(out=attsT[:, :], in_=ps_ta[:], func=AF.Copy)
                    # agg.T = Wm2aug.T @ [t.T ; attsum]
                    for mc in range(2):
                        ps_ag = psS.tile([128, 124], F32, tag="s")
                        ms = slice(mc * 128, (mc + 1) * 128)
                        nc.tensor.matmul(ps_ag[:], lhsT=wb[:, 2048:2304][:, ms], rhs=taugT[:, 0, :], start=True, stop=False)
                        nc.tensor.matmul(ps_ag[:], lhsT=wb[:, 2304:2560][:, ms], rhs=taugT[:, 1, :], start=False, stop=False)
                        nc.tensor.matmul(ps_ag[:], lhsT=wb[0:1, 2560:2816][:, ms],
                                         rhs=attsT[:, :], start=False, stop=True)
                        nc.scalar.activation(out=aggT[:, mc, w * WN : (w + 1) * WN], in_=ps_ag[:], func=AF.Copy)

                # ---------- GRU (batched over node chunks)
                IH, HH = 2816, 5120
                NCH = -(-NL // 400)
                while NL % NCH:
                    NCH += 1
                CHW = NL // NCH
                assert CHW * NCH == NL and CHW <= 512

                def gemm_gate(ps, ms, rhs_a, rhs_h, a_only=False, h_only=False):
                    first = True
                    for woff, r2 in ((IH, rhs_a), (HH, rhs_h)):
                        if (a_only and woff == HH) or (h_only and woff == IH):
                            continue
                        nc.tensor.matmul(ps[:], lhsT=wb[:, woff : woff + 768][:, ms], rhs=r2[0],
                                         start=first, stop=False, skip_group_check=True)
                        nc.tensor.matmul(ps[:], lhsT=wb[:, woff + 768 : woff + 1536][:, ms], rhs=r2[1],
                                         start=False, stop=False, skip_group_check=True)
                        last = (woff == HH) or a_only or h_only
                        nc.tensor.matmul(ps[:], lhsT=wb[0:1, woff + 1536 : woff + 2304][:, ms], rhs=r2[2],
                                         start=False, stop=last, skip_group_check=True)
                        first = False

                for ch in range(NCH):
                    nsl = slice(ch * CHW, (ch + 1) * CHW)
                    rhs_a = (aggT[:, 0, nsl], aggT[:, 1, nsl], ones[:, 0:CHW])
                    rhs_h = (hTb[:, 0, nsl], hTb[:, 1, nsl], ones[:, 0:CHW])
                    rz_sb = sml.tile([128, 4, CHW], BF16, tag="rz")
                    for mc in range(4):  # r,z rows 0:512, both sides accumulated
                        ps = psA.tile([128, CHW], F32, tag="gru")
                        gemm_gate(ps, slice(mc * 128, (mc + 1) * 128), rhs_a, rhs_h)
                        nc.scalar.activation(out=rz_sb[:, mc, :], in_=ps[:], func=AF.Sigmoid)
                    for fc in range(2):  # n rows 512:768
                        ms = slice(512 + fc * 128, 512 + (fc + 1) * 128)
                        ps_in = psA.tile([128, CHW], F32, tag="gru")
                        gemm_gate(ps_in, ms, rhs_a, rhs_h, a_only=True)
                        ps_hn = psA.tile([128, CHW], F32, tag="gru")
                        gemm_gate(ps_hn, ms, rhs_a, rhs_h, h_only=True)
                        nn_sb = sml.tile([128, CHW], F32, tag="nn")
                        # nn = tanh(in + r*hn)
                        nc.vector.tensor_tensor(out=nn_sb[:], in0=rz_sb[:, fc, :], in1=ps_hn[:], op=OP.mult)
                        nc.vector.tensor_tensor(out=nn_sb[:], in0=nn_sb[:], in1=ps_in[:], op=OP.add)
                        nc.scalar.activation(out=nn_sb[:], in_=nn_sb[:], func=AF.Tanh)
                        # h' = nn + z*(h - nn)
                        hmn = sml.tile([128, CHW], F32, tag="hmn")
                        nc.vector.tensor_tensor(out=hmn[:], in0=hT[:, fc, nsl], in1=nn_sb[:], op=OP.subtract)
                        nc.vector.tensor_tensor(out=hmn[:], in0=hmn[:], in1=rz_sb[:, 2 + fc, :], op=OP.mult)
                        nc.vector.tensor_tensor(out=hT[:, fc, nsl], in0=hmn[:], in1=nn_sb[:], op=OP.add)

            if dbg_h is not None:
                nc.sync.dma_start(out=dbg_h[:, :], in_=hT[:].rearrange("p c n -> p (c n)"))
            # ================= tail =================
            tb = per.tile([128, meta["small_cols"]], BF16, tag="tail")
            nc.sync.dma_start(out=tb[:], in_=tail_d[:, 0 : meta["small_cols"]])
            rb = per.tile([1, meta["row_cols"]], BF16, tag="row")
            nc.sync.dma_start(out=rb[:], in_=row_d[:])
            pidx_sb = sml.tile([128, 16], I16, tag="pidx")
            nc.sync.dma_start(out=pidx_sb[:], in_=pidx_d[:])

            # node-major h (bf16) for prev-node gather, via PE transposes
            nc.vector.tensor_copy(out=hTb[:].rearrange("p c n -> p (c n)"),
                                  in_=hT[:].rearrange("p c n -> p (c n)"))
            hnm_sb = wgt.tile([124, 2, 128], BF16, tag="hnm")
            for w in range(NW):
                for fc in range(2):
                    ps_tr = psS.tile([128, 128], BF16, tag="s")
                    nc.tensor.transpose(out=ps_tr[0:124, :], in_=hTb[:, fc, w * WN : (w + 1) * WN], identity=ident[:])
                    nc.scalar.activation(out=hnm_sb[:, fc, :], in_=ps_tr[0:124, :], func=AF.Copy)
                nc.sync.dma_start(out=hnm_in[w * WN : (w + 1) * WN, :],
                                  in_=hnm_sb[:].rearrange("p c d -> p (c d)"))
            # zero row for non-owned prev-node slots
            zrow = wgt.tile([16, 256], BF16, tag="hnm")
            nc.vector.memset(zrow[:], 0.0)
            nc.sync.dma_start(out=hnm_in[NL : NL + 16, :], in_=zrow[:])
            # local sums for hG
            hsum = sml.tile([128, 2], F32, tag="hsum")
            nc.vector.tensor_reduce(out=hsum[:], in_=hT[:], axis=AX.X, op=OP.add)
            nc.sync.dma_start(out=red_in[:, :], in_=hsum[:])
            if "nocc" not in ABL:
                nc.gpsimd.collective_compute("AllReduce", OP.add, replica_groups=grp,
                                             ins=[red_in[:, :].opt()], outs=[red_out[:, :].opt()])
            hG = sml.tile([128, 2], F32, tag="hG")
            nc.sync.dma_start(out=hG[:], in_=red_out[:, :])
            # gather the <=101 wanted rows from the LOCAL table, then merge
            # across cores with a tiny AllReduce (non-owners gathered zeros)
            gp = wgt.tile([128, 2, 256], BF16, tag="gp")
            nc.gpsimd.dma_gather(out_ap=gp[:], in_ap=hnm_in[:, :], idxs_ap=pidx_sb[:],
                                 num_idxs=256, num_idxs_reg=nidx_reg2, elem_size=256)
            nc.sync.dma_start(out=red2_in[:, :], in_=gp[:].rearrange("p t d -> p (t d)"))
            if "nocc" not in ABL:
                nc.gpsimd.collective_compute("AllReduce", OP.add, replica_groups=grp,
                                             ins=[red2_in[:, :].opt()], outs=[red2_out[:, :].opt()])
            nc.sync.dma_start(out=gp[:].rearrange("p t d -> p (t d)"), in_=red2_out[:, :])

            M2 = M + 1
            eiT = per.tile([128, 6, M2], BF16, tag="eiT")
            invN = 1.0 / cfg.N
            for fc in range(2):
                # hG rows (scaled by 1/N)
                nc.vector.tensor_scalar(out=eiT[:, fc, :], in0=hG[:, fc : fc + 1].to_broadcast([128, M2]),
                                        scalar1=invN, scalar2=None, op0=OP.mult)
                # t_emb rows: transpose gathered row M (target)
                ps_te = psS.tile([128, 1], BF16, tag="s")
                nc.tensor.transpose(out=ps_te[:], in_=gp[0:1, 1, fc * 128 : (fc + 1) * 128],
                                    identity=ident[0:1, 0:1])
                te = sml.tile([128, 1], BF16, tag="tecp")
                nc.scalar.activation(out=te[:], in_=ps_te[:], func=AF.Copy)
                nc.vector.tensor_scalar(out=eiT[:, 2 + fc, :], in0=te[:].to_broadcast([128, M2]),
                                        scalar1=1.0, scalar2=None, op0=OP.mult)
                # h_vj rows: transpose gathered rows 0:M
                ps_tv = psS.tile([128, M2], BF16, tag="s")
                nc.tensor.transpose(out=ps_tv[:, 0:M], in_=gp[0:M, 0, fc * 128 : (fc + 1) * 128],
                                    identity=ident[0:M, 0:M])
                nc.vector.memset(eiT[:, 4 + fc, :], 0.0)
                nc.scalar.activation(out=eiT[:, 4 + fc, 0:M], in_=ps_tv[:, 0:M], func=AF.Copy)

            def mlp_head(w1t, w1off, b1off, kchunks, outT):
                # outT [128, 2, M2] bf16 = relu(W1.T @ eiT + b1)
                for mc in range(2):
                    ps = psS.tile([128, M2], F32, tag="s")
                    for kc in range(kchunks):
                        nc.tensor.matmul(ps[:], lhsT=w1t[:, w1off + kc * 256 : w1off + kc * 256 + 256][:, mc * 128 : (mc + 1) * 128],
                                         rhs=eiT[:, kc, :], start=(kc == 0), stop=(kc == kchunks - 1))
                    nc.scalar.activation(out=outT[:, mc, :], in_=ps[:], func=AF.Relu,
                                         bias=tb[:, b1off + mc : b1off + mc + 1], scale=1.0)

            def softmax_free(x_sb, n_p, n_f):
                mx = sml.tile([n_p, 1], F32, tag="mx")
                nc.vector.tensor_reduce(out=mx[:], in_=x_sb[:], axis=AX.X, op=OP.max, negate=True)
                se = sml.tile([n_p, 1], F32, tag="se")
                nc.scalar.activation(out=x_sb[:], in_=x_sb[:], func=AF.Exp, bias=mx[:], scale=1.0, accum_out=se[:])
                nc.vector.reciprocal(out=se[:], in_=se[:])
                nc.vector.tensor_scalar(out=x_sb[:], in0=x_sb[:], scalar1=se[:], scalar2=None, op0=OP.mult)

            if dbg_ei is not None:
                nc.gpsimd.dma_start(out=dbg_ei[:, :], in_=eiT[:].rearrange("p c n -> p (c n)"))
            # ---- mix head
            mixT = sml.tile([128, 2, M2], BF16, tag="mixT")
            mlp_head(tb, toffs["mW1"], toffs["mb1"], 6, mixT)
            mixl = sml.tile([M2, K], F32, tag="mixl")
            ps_ml = psS.tile([M2, K], F32, tag="s")
            for mc in range(2):
                nc.tensor.matmul(ps_ml[:], lhsT=mixT[:, mc, :], rhs=tb[:, toffs["mW2"] + mc * K : toffs["mW2"] + (mc + 1) * K],
                                 start=(mc == 0), stop=False, skip_group_check=True)
            nc.tensor.matmul(ps_ml[:], lhsT=ones[:, 0:M2], rhs=rb[:, roffs["mb2"] : roffs["mb2"] + K],
                             start=False, stop=True, skip_group_check=True)
            nc.scalar.activation(out=mixl[:], in_=ps_ml[:], func=AF.Copy)
            softmax_free(mixl, M2, K)

            if dbg_mix is not None:
                nc.sync.dma_start(out=dbg_mix[:, :], in_=mixl[:])
            # ---- edge heads
            acc = sml.tile([M2, ET], F32, tag="acc")
            nc.vector.memset(acc[:], 0.0)
            hkT = sml.tile([128, 2, M2], BF16, tag="hkT")
            for k in range(K):
                ekw = wgt.tile([128, 6 * 256], BF16, tag="ekw")
                nc.sync.dma_start(out=ekw[:], in_=tail_d[:, toffs["eW1"] + k * 6 * 256 : toffs["eW1"] + (k + 1) * 6 * 256])
                mlp_head(ekw, 0, toffs["eb1"] + k * 2, 6, hkT)
                ps_el = psS.tile([M2, ET], F32, tag="s")
                for mc in range(2):
                    nc.tensor.matmul(ps_el[:], lhsT=hkT[:, mc, :],
                                     rhs=tb[:, toffs["eW2"] + (k * 2 + mc) * ET : toffs["eW2"] + (k * 2 + mc + 1) * ET],
                                     start=(mc == 0), stop=False, skip_group_check=True)
                nc.tensor.matmul(ps_el[:], lhsT=ones[:, 0:M2],
                                 rhs=rb[:, roffs["eb2"] + k * ET : roffs["eb2"] + (k + 1) * ET],
                                 start=False, stop=True, skip_group_check=True)
                ekl = sml.tile([M2, ET], F32, tag="ekl")
                nc.scalar.activation(out=ekl[:], in_=ps_el[:], func=AF.Copy)
                softmax_free(ekl, M2, ET)
                nc.vector.scalar_tensor_tensor(out=acc[:], in0=ekl[:], scalar=mixl[:, k : k + 1],
                                               in1=acc[:], op0=OP.mult, op1=OP.add)
            nc.sync.dma_start(out=eprob_d[:, :], in_=acc[0:M, :])

            # ---- node head
            niT = sml.tile([128, 4], BF16, tag="niT")
            for fc in range(2):
                nc.vector.tensor_scalar(out=niT[:, fc : fc + 1], in0=hG[:, fc : fc + 1],
                                        scalar1=invN, scalar2=None, op0=OP.mult)
                nc.vector.tensor_copy(out=niT[:, 2 + fc : 3 + fc], in_=eiT[:, 2 + fc, 0:1])
            nhT = sml.tile([128, 2], BF16, tag="nhT")
            for mc in range(2):
                ps = psS.tile([128, 1], F32, tag="s")
                for kc in range(4):
                    nc.tensor.matmul(ps[:], lhsT=tb[:, toffs["nW1"] + kc * 256 : toffs["nW1"] + kc * 256 + 256][:, mc * 128 : (mc + 1) * 128],
                                     rhs=niT[:, kc : kc + 1], start=(kc == 0), stop=(kc == 3))
                nc.scalar.activation(out=nhT[:, mc : mc + 1], in_=ps[:], func=AF.Relu,
                                     bias=tb[:, toffs["nb1"] + mc : toffs["nb1"] + mc + 1], scale=1.0)
            ps_nl = psS.tile([1, NT], F32, tag="s")
            for mc in range(2):
                nc.tensor.matmul(ps_nl[:], lhsT=nhT[:, mc : mc + 1],
                                 rhs=tb[:, toffs["nW2"] + mc * NT : toffs["nW2"] + (mc + 1) * NT],
                                 start=(mc == 0), stop=False, skip_group_check=True)
            nc.tensor.matmul(ps_nl[:], lhsT=ones[:, 0:1], rhs=rb[:, roffs["nb2"] : roffs["nb2"] + NT],
                             start=False, stop=True, skip_group_check=True)
            nl_sb = sml.tile([1, NT], F32, tag="nlsb")
            nc.scalar.activation(out=nl_sb[:], in_=ps_nl[:], func=AF.Copy)
            softmax_free(nl_sb, 1, NT)
            nc.sync.dma_start(out=nprob_d[:, :], in_=nl_sb[:])

    nc.compile()
    return nc


# --------------------------------------------------------------------------
_LAST_RESULTS = {}


def _run_spmd_timed(nc, in_maps, n_cores, reps=3):
    """Mirror bass2jax.run_bass_via_pjrt's multi-core path, but jit once,
    device_put inputs once, and wall-clock steady-state executions."""
    import time
    import jax
    from jax.sharding import Mesh, PartitionSpec, NamedSharding
    from jax.experimental.shard_map import shard_map
    import concourse.mybir as mybir_
    from concourse import bass2jax

    bass2jax.install_neuronx_cc_hook()
    partition_name = nc.partition_id_tensor.name if nc.partition_id_tensor else None
    in_names, out_names, out_avals, zero_shapes = [], [], [], []
    for alloc in nc.m.functions[0].allocations:
        if not isinstance(alloc, mybir_.MemoryLocationSet):
            continue
        name = alloc.memorylocations[0].name
        if alloc.kind == "ExternalInput":
            if name != partition_name:
                in_names.append(name)
        elif alloc.kind == "ExternalOutput":
            out_names.append(name)
            shape = tuple(alloc.tensor_shape)
            dtype = mybir_.dt.np(alloc.dtype)
            out_avals.append(jax.core.ShapedArray(shape, dtype))
            zero_shapes.append((shape, dtype))
    n_params = len(in_names)
    n_outs = len(out_avals)
    all_names = list(in_names) + out_names + ([partition_name] if partition_name else [])
    donate = tuple(range(n_params, n_params + n_outs))

    def _body(*args):
        operands = list(args)
        if partition_name is not None:
            operands.append(bass2jax.partition_id_tensor())
        return tuple(bass2jax._bass_exec_p.bind(
            *operands, out_avals=tuple(out_avals), in_names=tuple(all_names),
            out_names=tuple(out_names), lowering_input_output_aliases=(),
            sim_require_finite=True, sim_require_nnan=True, nc=nc))

    devices = jax.devices()[:n_cores]
    mesh = Mesh(np.asarray(devices), ("core",))
    spec = PartitionSpec("core")
    sharded = jax.jit(
        shard_map(_body, mesh=mesh, in_specs=(spec,) * (n_params + n_outs),
                  out_specs=(spec,) * n_outs, check_rep=False),
        donate_argnums=donate, keep_unused=True)
    concat_in = [
        np.concatenate([np.asarray(in_maps[c][nm]) for c in range(n_cores)], axis=0)
        for nm in in_names
    ]
    sh = NamedSharding(mesh, spec)
    dev_in = [jax.device_put(a, sh) for a in concat_in]

    def zeros():
        return [jax.device_put(np.zeros((n_cores * s[0], *s[1:]), d), sh)
                for s, d in zero_shapes]

    outs = sharded(*dev_in, *zeros())
    jax.block_until_ready(outs)
    times = []
    for _ in range(reps):
        z = zeros()
        jax.block_until_ready(z)
        t0 = time.perf_counter()
        outs = sharded(*dev_in, *z)
        jax.block_until_ready(outs)
        times.append(time.perf_counter() - t0)
    results = [
        {nm: np.asarray(outs[i]).reshape(n_cores, *zero_shapes[i][0])[c]
         for i, nm in enumerate(out_names)}
        for c in range(n_cores)
    ]
    return results, min(times)


def kernel(**inputs):
    from concourse.bass_utils import run_bass_kernel_spmd

    C = 8
    in_maps, meta = preprocess(inputs, C=C)
    nc = build(meta)
    if os.environ.get("BASSGNN_TIME", "0") == "1":
        results, secs = _run_spmd_timed(nc, in_maps, C)
        _LAST_RESULTS["exec_time_ns"] = secs * 1e9
        node_probs = np.asarray(results[0]["node_probs"]).reshape(-1)
        edge_probs = np.asarray(results[0]["edge_probs"])
        return node_probs, edge_probs
    trace = os.environ.get("BASSGNN_TRACE", "0") == "1"
    res = run_bass_kernel_spmd(nc, in_maps, core_ids=list(range(C)), trace=trace)
    _LAST_RESULTS["res"] = res
    node_probs = np.asarray(res.results[0]["node_probs"]).reshape(-1)
    edge_probs = np.asarray(res.results[0]["edge_probs"])
    return node_probs, edge_probs
